# revision 22
# baseline (speedup 1.0000x reference)
"""Tensor-parallel GQA attention prefill for 8 TRN2 NeuronCores.

Sharding: each core owns 4 query heads + 1 kv head (column-shard of
wq/wk/wv by head) and a 512-row slice of wo's input dim (row-shard).
Each core computes a partial output projection over its local heads;
the host sums the 8 partials (equivalent to the all-reduce in the
sharding hint) and transposes back to [b, s, d].

Fast-path schedule, built from the measured baseline trace:
  - One combined PE sweep accumulates K, V, Q0, Q1, Q2 over the 32
    d-tiles; Q3's sweep runs afterwards, interleaved with the RoPE /
    score chains of the earlier heads so the PE never idles (an idle
    PE drops the HAM duty cycle to 4/8 for ~10us -- half-speed
    matmuls afterwards).
  - Inputs stream in consumption order, chunked, on the sync+gpsimd
    DMA rings (the scalar ring starts ~3us late on hw and only
    carries late-needed tensors). The first matmul needs only ~400KB.
  - Softmax: causal mask is preloaded into PSUM by the DVE, scores
    accumulate on top, exp reads PSUM directly with no reduce_max
    (logits are bounded in the smooth regime), so the per-head chain
    is short.
  - The wo projection is chunked per d-tile; the first chunks are
    interleaved between the PV matmuls to cover the softmax chains.
  - y is staged 4 d-tiles at a time and written with large DMAs to a
    pre-swizzled [128, ND*T] output (host unswizzles).

The robust (fp32r) path for winner-take-all softmax inputs keeps the
original proven build.
"""

import math
from contextlib import ExitStack

import ml_dtypes
import numpy as np

import concourse.bass as bass
import concourse.tile as tile
from concourse import bacc, mybir
from concourse.bass_utils import run_bass_kernel_spmd

DIM = 4096
N_HEADS = 32
HEAD_DIM = 128
N_KV_HEADS = 8
BSZ = 4
SEQLEN = 128
T = BSZ * SEQLEN  # 512 tokens
NCORES = 8
HQ = N_HEADS // NCORES  # 4 query heads per core
EQ = HQ * HEAD_DIM  # 512 local q features
ND = DIM // 128  # 32 contraction tiles
SCALE = 1.0 / math.sqrt(HEAD_DIM)

F32 = mybir.dt.float32
F32R = mybir.dt.float32r
BF16 = mybir.dt.bfloat16
AX = mybir.AxisListType
ACTF = mybir.ActivationFunctionType
PSUM = bass.MemorySpace.PSUM

_STATE: dict = {}
LAST_RESULT = None


def _install_ntff_hook():
    """Register the axon NTFF profile hook if the image lacks antenv.axon_hooks.

    Lets run_bass_kernel_spmd(trace=True) return exec_time_ns + perfetto
    under axon. Best-effort: any failure leaves tracing disabled but the
    kernel still runs.
    """
    import os
    import sys
    import types

    try:
        import antenv.axon_hooks  # noqa: F401

        return
    except ImportError:
        pass
    try:
        import antenv
        from trn_agent_boot.trn_boot import _ntff_profile_via_ctypes

        mod = types.ModuleType("antenv.axon_hooks")
        holder = {"hook": None}
        mod.set_axon_ntff_profile_hook = lambda h: holder.__setitem__("hook", h)
        mod.get_axon_ntff_profile_hook = lambda: holder["hook"]
        sys.modules["antenv.axon_hooks"] = mod
        antenv.axon_hooks = mod
        so = "/opt/axon/libaxon_pjrt.so"
        if os.path.exists(so):
            hook = _ntff_profile_via_ctypes(so)
            if hook is not None:
                mod.set_axon_ntff_profile_hook(hook)
    except Exception:
        pass


_install_ntff_hook()

# x-group boundaries (d-tiles): even groups stream on the sync ring,
# odd groups on the gpsimd ring.
XG = [(0, 2), (2, 4), (4, 7), (7, 10), (10, 14), (14, 18), (18, 22),
      (22, 26), (26, 29), (29, 32)]
# weight chunk boundaries (d-tiles), shared by wk/wv/wq0..2
WCH = [(0, 2), (2, 5), (5, 10), (10, 16), (16, 24), (24, 32)]


def _build_nc_fast():
    """Build the bf16 SPMD kernel graph (smooth-softmax regime)."""
    XD = BF16
    QD = BF16
    TD = BF16
    nc = bacc.Bacc(
        "TRN2",
        target_bir_lowering=False,
        debug=False,
        enable_asserts=False,
        num_devices=NCORES,
    )
    xT = nc.dram_tensor("xT", [128, ND * T], XD, kind="ExternalInput").ap()
    wqT = nc.dram_tensor("wqT", [128, HQ * ND * HEAD_DIM], XD, kind="ExternalInput").ap()
    wkT = nc.dram_tensor("wkT", [128, ND * HEAD_DIM], XD, kind="ExternalInput").ap()
    wvT = nc.dram_tensor("wvT", [128, ND * HEAD_DIM], XD, kind="ExternalInput").ap()
    woT = nc.dram_tensor("woT", [128, HQ * DIM], BF16, kind="ExternalInput").ap()
    # causal mask tiled 4x along tokens, bf16 (preloaded into PSUM via PE)
    maskb = nc.dram_tensor("maskb", [128, T], BF16, kind="ExternalInput").ap()
    # tables: ck | sk | cq | sq, each [128, T]
    tabs = nc.dram_tensor("tabs", [128, 4 * T], TD, kind="ExternalInput").ap()
    ident = nc.dram_tensor("ident", [128, 128], BF16, kind="ExternalInput").ap()
    yT = nc.dram_tensor("yT", [128, ND * T], BF16, kind="ExternalOutput").ap()

    with tile.TileContext(nc) as tc, ExitStack() as ctx:
        const = ctx.enter_context(tc.tile_pool(name="const", bufs=1))
        wp = ctx.enter_context(tc.tile_pool(name="wp", bufs=6))
        qtp = ctx.enter_context(tc.tile_pool(name="qtp", bufs=4))
        rt = ctx.enter_context(tc.tile_pool(name="rt", bufs=2))
        sm = ctx.enter_context(tc.tile_pool(name="sm", bufs=3))
        yp = ctx.enter_context(tc.tile_pool(name="yp", bufs=3))
        ps = ctx.enter_context(tc.tile_pool(name="ps", bufs=6, space=PSUM))
        psy = ctx.enter_context(tc.tile_pool(name="psy", bufs=2, space=PSUM))

        # ---- warm tiles (memset on gpsimd before its DMA issues) ----
        warm_w = const.tile([128, 128], BF16, tag="warm_w")
        nc.gpsimd.memset(warm_w[:], 0.0)
        warm_x = const.tile([128, T], BF16, tag="warm_x")
        nc.gpsimd.memset(warm_x[:], 0.0)

        # ---- SBUF tiles fed by DMA ----
        wk_sb = wp.tile([128, ND * HEAD_DIM], XD, tag="w", name="wk")
        wv_sb = wp.tile([128, ND * HEAD_DIM], XD, tag="w", name="wv")
        wq_tiles = [
            wp.tile([128, ND * HEAD_DIM], XD, tag="w", name=f"wq{h}")
            for h in range(3)
        ]
        x_tiles = [None] * len(XG)

        def load_x(gi, eng):
            j0, j1 = XG[gi]
            xg = const.tile([128, (j1 - j0) * T], XD, tag=f"x{gi}", name=f"x{gi}")
            eng.dma_start(xg[:], xT[:, j0 * T : j1 * T])
            x_tiles[gi] = xg

        def load_wch(eng, sb, dram, base, c0, c1):
            # chunk [c0:c1) d-tiles of a [128, ND*HEAD_DIM] weight tile
            eng.dma_start(
                sb[:, c0 * HEAD_DIM : c1 * HEAD_DIM],
                dram[:, base + c0 * HEAD_DIM : base + c1 * HEAD_DIM],
            )

        tabs_sb = const.tile([128, 4 * T], TD, tag="tabs")
        ident_sb = const.tile([128, 128], BF16, tag="ident")
        mask_sb = const.tile([128, T], BF16, tag="mask")
        wq3_sb = wp.tile([128, ND * HEAD_DIM], XD, tag="w", name="wq3")

        # ---- DMA issue order ----
        # sync ring: wk+wq0 chunks and even x groups (consumption order),
        # then half of wo. gpsimd ring: wv+wq1+wq2 chunks, odd x groups,
        # other half of wo, y outputs later. scalar ring: wq3 + tables +
        # ident/mask, gated behind a dummy read of a mid-sweep x group so
        # they transfer only once the critical stream has headroom.
        def sync_w(c):
            c0, c1 = WCH[c]
            load_wch(nc.sync, wk_sb, wkT, 0, c0, c1)
            load_wch(nc.sync, wq_tiles[0], wqT, 0 * DIM, c0, c1)

        def gp_w(c):
            c0, c1 = WCH[c]
            load_wch(nc.gpsimd, wv_sb, wvT, 0, c0, c1)
            load_wch(nc.gpsimd, wq_tiles[1], wqT, 1 * DIM, c0, c1)
            load_wch(nc.gpsimd, wq_tiles[2], wqT, 2 * DIM, c0, c1)

        sync_w(0)
        load_x(0, nc.sync)
        gp_w(0)
        load_x(1, nc.gpsimd)
        sync_w(1)
        load_x(2, nc.sync)
        gp_w(1)
        sync_w(2)
        load_x(4, nc.sync)
        gp_w(2)
        load_x(3, nc.gpsimd)
        sync_w(3)
        gp_w(3)
        load_x(5, nc.gpsimd)
        sync_w(4)
        load_x(6, nc.sync)
        gp_w(4)
        load_x(7, nc.gpsimd)
        sync_w(5)
        load_x(8, nc.sync)
        gp_w(5)
        load_x(9, nc.gpsimd)
        wo_sb = const.tile([128, HQ * DIM], BF16, tag="wo")
        nc.sync.dma_start(wo_sb[:, : 2 * DIM], woT[:, : 2 * DIM])
        nc.gpsimd.dma_start(wo_sb[:, 2 * DIM :], woT[:, 2 * DIM :])
        # scalar ring, gated: wait for xg7 (~mid-sweep) before issuing
        gate = const.tile([128, 1], BF16, tag="gate")
        nc.scalar.copy(gate[:], x_tiles[7][:, 0:1])
        nc.scalar.dma_start(wq3_sb[:, : 16 * HEAD_DIM], wqT[:, 3 * DIM : 3 * DIM + 2048])
        nc.scalar.dma_start(wq3_sb[:, 16 * HEAD_DIM :], wqT[:, 3 * DIM + 2048 : 4 * DIM])
        nc.scalar.dma_start(tabs_sb[:], tabs)
        nc.scalar.dma_start(ident_sb[:], ident)
        nc.scalar.dma_start(mask_sb[:], maskb)

        ck_t = tabs_sb[:, 0:T]
        sk_t = tabs_sb[:, T : 2 * T]
        cq_t = tabs_sb[:, 2 * T : 3 * T]
        sq_t = tabs_sb[:, 3 * T : 4 * T]

        kT_sb = const.tile([128, T], QD, tag="kT")
        vT_sb = const.tile([128, T], BF16, tag="vT")
        v_sb = const.tile([128, BSZ * HEAD_DIM], BF16, tag="v")
        oT_sb = const.tile([128, HQ * T], BF16, tag="oT")

        def xslice(j):
            for gi, (j0, j1) in enumerate(XG):
                if j0 <= j < j1:
                    return x_tiles[gi][:, (j - j0) * T : (j - j0 + 1) * T]
            raise AssertionError(j)

        # ---- PE warm-up: dummy matmuls bridge until the first data
        # lands (~12.5us), lifting the HAM clock gate on the way ----
        ps_warm = ps.tile([128, T], F32, tag="ps", name="warm")
        for _ in range(12):
            nc.tensor.matmul(ps_warm[:], warm_w[:], warm_x[:], start=True, stop=True)
        for _ in range(6):
            nc.tensor.matmul(
                ps_warm[:, 0:128], warm_w[:], warm_x[:, 0:128], start=True, stop=True
            )

        def rope(dst_ap, pssrc, ctab, stab, evict=None):
            # muls read the PSUM source directly (PSUM+SB operand mix is
            # exempt from the equal-base-partition rule), so no eviction
            # pass is needed and the half-swap folds into the sin-muls.
            swp = rt.tile([128, T], BF16, tag="swp")
            nc.vector.tensor_mul(swp[0:64, :], pssrc[64:128, :], stab[0:64, :])
            nc.vector.tensor_mul(swp[64:128, :], pssrc[0:64, :], stab[64:128, :])
            prod = rt.tile([128, T], BF16, tag="prod")
            nc.vector.tensor_mul(prod[:], pssrc[:], ctab)
            nc.vector.tensor_add(dst_ap, prod[:], swp[:])

        # ---- combined sweep: K, V, Q0, Q1, Q2 accumulate together over
        # the 32 d-tiles, riding the incoming x stream ----
        ps_k = ps.tile([128, T], F32, tag="ps", name="ps_k")
        ps_v = ps.tile([128, T], F32, tag="ps", name="ps_v")
        ps_q = [None] * HQ
        for h in range(3):
            ps_q[h] = ps.tile([128, T], F32, tag="ps", name=f"ps_q{h}")
        for j in range(ND):
            st, sp = (j == 0), (j == ND - 1)
            xr = xslice(j)
            js = slice(j * HEAD_DIM, (j + 1) * HEAD_DIM)
            nc.tensor.matmul(ps_k[:], wk_sb[:, js], xr, start=st, stop=sp)
            nc.tensor.matmul(ps_v[:], wv_sb[:, js], xr, start=st, stop=sp)
            for h in range(3):
                nc.tensor.matmul(ps_q[h][:], wq_tiles[h][:, js], xr, start=st, stop=sp)

        # rope chains drain PSUM on the DVE; vT eviction on scalar
        rope(kT_sb[:], ps_k[:], ck_t, sk_t)
        nc.scalar.copy(vT_sb[:], ps_v[:])
        qts = {}
        for h in range(3):
            qts[h] = qtp.tile([128, T], QD, tag="qT", name=f"qT{h}")
            rope(qts[h][:], ps_q[h][:], cq_t, sq_t)

        # ---- Q3 sweep, interleaved with attention entry ----
        ps_q3 = ps.tile([128, T], F32, tag="ps", name="ps_q3")

        def q3s(j0, j1):
            for j in range(j0, j1):
                nc.tensor.matmul(
                    ps_q3[:],
                    wq3_sb[:, j * HEAD_DIM : (j + 1) * HEAD_DIM],
                    xslice(j),
                    start=(j == 0),
                    stop=(j == ND - 1),
                )

        def v_transpose():
            for b in range(BSZ):
                bs = slice(b * 128, (b + 1) * 128)
                ps_t = ps.tile([128, T], BF16, tag="ps", name=f"ps_vt{b}")
                nc.tensor.transpose(ps_t[:, 0:128], vT_sb[:, bs], ident_sb[:])
                nc.vector.tensor_copy(v_sb[:, bs], ps_t[:, 0:128])

        s_tiles = {}
        den = sm.tile([128, BSZ * HQ], F32, tag="den")
        rden = sm.tile([128, BSZ * HQ], F32, tag="rden")

        def att_scores(h, qt):
            # mask preloaded into PSUM via identity matmul; scores
            # accumulate on top (start=False)
            ps_s = ps.tile([128, T], F32, tag="ps", name=f"ps_s{h}")
            nc.tensor.matmul(ps_s[:], ident_sb[:], mask_sb[:], start=True, stop=False)
            for b in range(BSZ):
                bs = slice(b * 128, (b + 1) * 128)
                nc.tensor.matmul(
                    ps_s[:, bs], qt[:, bs], kT_sb[:, bs], start=False, stop=True
                )
            s_tiles[h] = ps_s

        def att_soft(h):
            # no reduce_max: logits are bounded in the smooth regime.
            # single exp instruction per head; den via reduce_sum on the
            # idle gpsimd engine.
            ps_s = s_tiles[h]
            hh = slice(h * BSZ, (h + 1) * BSZ)
            p_sb = sm.tile([128, T], BF16, tag="p", name=f"p{h}")
            nc.scalar.activation(p_sb[:], ps_s[:], ACTF.Exp)
            for b in range(BSZ):
                bs = slice(b * 128, (b + 1) * 128)
                nc.vector.reduce_sum(
                    den[:, h * BSZ + b : h * BSZ + b + 1], p_sb[:, bs], axis=AX.X
                )
            nc.vector.reciprocal(rden[:, hh], den[:, hh])
            # normalization muls on the otherwise-idle gpsimd engine
            for b in range(BSZ):
                bs = slice(b * 128, (b + 1) * 128)
                nc.gpsimd.tensor_scalar_mul(
                    p_sb[:, bs], p_sb[:, bs], rden[:, h * BSZ + b : h * BSZ + b + 1]
                )
            return p_sb

        pts = {}

        def att_ptrans(h, p_sb):
            ps_pt = ps.tile([128, T], BF16, tag="ps", name=f"ps_pt{h}")
            for b in range(BSZ):
                bs = slice(b * 128, (b + 1) * 128)
                nc.tensor.transpose(ps_pt[:, bs], p_sb[:, bs], ident_sb[:])
            pt_sb = sm.tile([128, T], BF16, tag="pt", name=f"pt{h}")
            if h % 2 == 0:
                nc.vector.tensor_copy(pt_sb[:], ps_pt[:])
            else:
                nc.scalar.copy(pt_sb[:], ps_pt[:])
            pts[h] = pt_sb

        def att_pv(h):
            ps_o = ps.tile([128, T], F32, tag="ps", name=f"ps_o{h}")
            for b in range(BSZ):
                bs = slice(b * 128, (b + 1) * 128)
                nc.tensor.matmul(
                    ps_o[:, bs], v_sb[:, bs], pts[h][:, bs], start=True, stop=True
                )
            if h % 2 == 0:
                nc.vector.tensor_copy(oT_sb[:, h * T : (h + 1) * T], ps_o[:])
            else:
                nc.scalar.copy(oT_sb[:, h * T : (h + 1) * T], ps_o[:])

        # ---- wo projection, chunked per d-tile ----
        ps_ys = {}

        def wo_mm(dt, j):
            if j == 0:
                # dt>=8: rotate through the main ps pool too (its att
                # tiles are gone by then), deepening the drain pipeline
                pool = psy if dt < 8 or dt % 2 == 0 else ps
                tg = "psy" if pool is psy else "ps"
                ps_ys[dt] = pool.tile([128, T], F32, tag=tg, name=f"ps_y{dt}")
            nc.tensor.matmul(
                ps_ys[dt][:],
                wo_sb[:, j * DIM + dt * 128 : j * DIM + (dt + 1) * 128],
                oT_sb[:, j * T : (j + 1) * T],
                start=(j == 0),
                stop=(j == HQ - 1),
            )

        # y staging: groups of 4 d-tiles (last two groups of 2)
        YG = [(0, 4), (4, 8), (8, 12), (12, 16), (16, 20), (20, 24),
              (24, 28), (28, 30), (30, 32)]
        ystage = {}

        def y_drain(dt):
            for g0, g1 in YG:
                if g0 <= dt < g1:
                    break
            if dt == g0:
                ystage[g0] = yp.tile([128, (g1 - g0) * T], BF16, tag="y", name=f"y{g0}")
            # split the drain across vector+scalar so its latency stays
            # below the 4-matmul chunk time (psy ping-pong never stalls)
            dst = ystage[g0][:, (dt - g0) * T : (dt - g0 + 1) * T]
            H = T // 2
            nc.vector.tensor_copy(dst[:, 0:H], ps_ys[dt][:, 0:H])
            nc.scalar.copy(dst[:, H:T], ps_ys[dt][:, H:T])
            del ps_ys[dt]
            if dt == g1 - 1:
                nc.gpsimd.dma_start(yT[:, g0 * T : g1 * T], ystage[g0][:])

        # ---- attention entry, interleaved with the q3 sweep ----
        q3s(0, 8)
        v_transpose()
        att_scores(0, qts[0])
        p0 = att_soft(0)
        q3s(8, 16)
        att_scores(1, qts[1])
        p1 = att_soft(1)
        q3s(16, 24)
        att_scores(2, qts[2])
        p2 = att_soft(2)
        q3s(24, ND)
        qts[3] = qtp.tile([128, T], QD, tag="qT", name="qT3")
        rope(qts[3][:], ps_q3[:], cq_t, sq_t, evict="vector")
        att_ptrans(0, p0)
        att_pv(0)
        att_ptrans(1, p1)
        att_pv(1)
        att_scores(3, qts[3])
        p3 = att_soft(3)
        wo_mm(0, 0)
        wo_mm(1, 0)
        wo_mm(0, 1)
        wo_mm(1, 1)
        att_ptrans(2, p2)
        att_pv(2)
        wo_mm(0, 2)
        wo_mm(1, 2)
        att_ptrans(3, p3)
        att_pv(3)
        wo_mm(0, 3)
        wo_mm(1, 3)
        y_drain(0)
        y_drain(1)
        for dt in range(2, ND):
            for j in range(HQ):
                wo_mm(dt, j)
            y_drain(dt)

    nc.compile()
    return nc


def _build_nc_robust():
    """Original proven build for the fp32r (winner-take-all) path."""
    XD = F32R
    QD = F32
    TD = F32
    nc = bacc.Bacc(
        "TRN2",
        target_bir_lowering=False,
        debug=False,
        enable_asserts=False,
        num_devices=NCORES,
    )
    xT = nc.dram_tensor("xT", [128, ND * T], XD, kind="ExternalInput").ap()
    wqT = nc.dram_tensor("wqT", [128, HQ * ND * HEAD_DIM], XD, kind="ExternalInput").ap()
    wkT = nc.dram_tensor("wkT", [128, ND * HEAD_DIM], XD, kind="ExternalInput").ap()
    wvT = nc.dram_tensor("wvT", [128, ND * HEAD_DIM], XD, kind="ExternalInput").ap()
    woT = nc.dram_tensor("woT", [128, HQ * DIM], BF16, kind="ExternalInput").ap()
    mask1 = nc.dram_tensor("mask1", [128, 128], F32, kind="ExternalInput").ap()
    cq = nc.dram_tensor("cq", [128, T], TD, kind="ExternalInput").ap()
    sq = nc.dram_tensor("sq", [128, T], TD, kind="ExternalInput").ap()
    ck = nc.dram_tensor("ck", [128, T], TD, kind="ExternalInput").ap()
    sk = nc.dram_tensor("sk", [128, T], TD, kind="ExternalInput").ap()
    ident = nc.dram_tensor("ident", [128, 128], BF16, kind="ExternalInput").ap()
    yT = nc.dram_tensor("yT", [DIM, T], BF16, kind="ExternalOutput").ap()

    with tile.TileContext(nc) as tc, ExitStack() as ctx:
        const = ctx.enter_context(tc.tile_pool(name="const", bufs=1))
        wp = ctx.enter_context(tc.tile_pool(name="wp", bufs=4))
        qtp = ctx.enter_context(tc.tile_pool(name="qtp", bufs=4))
        rt = ctx.enter_context(tc.tile_pool(name="rt", bufs=1))
        sm = ctx.enter_context(tc.tile_pool(name="sm", bufs=2))
        yp = ctx.enter_context(tc.tile_pool(name="yp", bufs=2))
        ps = ctx.enter_context(tc.tile_pool(name="ps", bufs=7, space=PSUM))
        wps = ctx.enter_context(tc.tile_pool(name="wps", bufs=1, space=PSUM))

        warm_w = const.tile([128, 128], BF16, tag="warm_w")
        nc.gpsimd.memset(warm_w[:], 0.0)
        warm_x = const.tile([128, T], BF16, tag="warm_x")
        nc.gpsimd.memset(warm_x[:], 0.0)
        ps_warm = wps.tile([128, T], F32, tag="wps")
        for _ in range(10):
            nc.tensor.matmul(ps_warm[:], warm_w[:], warm_x[:], start=True, stop=True)

        wk_sb = wp.tile([128, ND * HEAD_DIM], XD, tag="w", name="wk")
        nc.sync.dma_start(wk_sb[:], wkT)
        wv_sb = wp.tile([128, ND * HEAD_DIM], XD, tag="w", name="wv")
        nc.scalar.dma_start(wv_sb[:], wvT)

        XGROUPS = [2, 2, 2, 2, 4, 4, 4, 4, 4, 4]
        x_tiles = [None] * len(XGROUPS)
        xg_col = []
        j0 = 0
        for gi, gd in enumerate(XGROUPS):
            xg_col.append((j0, gd))
            j0 += gd

        def load_x(gi, eng):
            j0, gd = xg_col[gi]
            xg = const.tile([128, gd * T], XD, tag=f"x{gi}", name=f"x{gi}")
            eng.dma_start(xg[:], xT[:, j0 * T : (j0 + gd) * T])
            x_tiles[gi] = xg

        wq_tiles = [None] * HQ

        def load_wq(h, eng):
            wqt = wp.tile([128, ND * HEAD_DIM], XD, tag="w", name=f"wq{h}")
            eng.dma_start(wqt[:], wqT[:, h * DIM : (h + 1) * DIM])
            wq_tiles[h] = wqt

        load_x(0, nc.sync)
        load_wq(0, nc.scalar)
        load_wq(1, nc.sync)
        load_x(1, nc.scalar)
        load_wq(2, nc.sync)
        load_x(2, nc.scalar)
        load_wq(3, nc.sync)
        ident_sb = const.tile([128, 128], BF16, tag="ident")
        nc.scalar.dma_start(ident_sb[:], ident)
        ck_sb = const.tile([128, T], TD, tag="ck")
        nc.scalar.dma_start(ck_sb[:], ck)
        sk_sb = const.tile([128, T], TD, tag="sk")
        nc.scalar.dma_start(sk_sb[:], sk)
        cq_sb = const.tile([128, T], TD, tag="cq")
        nc.scalar.dma_start(cq_sb[:], cq)
        sq_sb = const.tile([128, T], TD, tag="sq")
        nc.scalar.dma_start(sq_sb[:], sq)
        mask_sb = const.tile([128, 128], F32, tag="mask")
        nc.scalar.dma_start(mask_sb[:], mask1)
        for gi in range(3, len(XGROUPS)):
            load_x(gi, nc.scalar if gi % 2 == 0 else nc.sync)
        wo_sb = const.tile([128, HQ * DIM], BF16, tag="wo")
        nc.sync.dma_start(wo_sb[:, : 2 * DIM], woT[:, : 2 * DIM])
        nc.scalar.dma_start(wo_sb[:, 2 * DIM :], woT[:, 2 * DIM :])

        kT_sb = const.tile([128, T], QD, tag="kT")
        vT_sb = const.tile([128, T], BF16, tag="vT")
        v_sb = const.tile([128, BSZ * HEAD_DIM], BF16, tag="v")
        oT_sb = const.tile([128, HQ * T], BF16, tag="oT")

        def xslice(j):
            gi = 0
            j0 = 0
            for i, (jj0, gd) in enumerate(xg_col):
                if jj0 <= j < jj0 + gd:
                    gi, j0 = i, jj0
                    break
            return x_tiles[gi][:, (j - j0) * T : (j - j0 + 1) * T]

        def rope(dst_ap, pssrc, ctab, stab):
            swp = rt.tile([128, T], F32, tag="swp")
            nc.scalar.copy(swp[0:64, :], pssrc[64:128, :])
            nc.scalar.copy(swp[64:128, :], pssrc[0:64, :])
            prod = rt.tile([128, T], F32, tag="prod")
            nc.vector.tensor_mul(prod[:], pssrc[:], ctab)
            nc.vector.tensor_mul(swp[:], swp[:], stab)
            nc.vector.tensor_add(dst_ap, prod[:], swp[:])

        ps_k = ps.tile([128, T], F32, tag="ps")
        ps_v = ps.tile([128, T], F32, tag="ps")
        ps_q = [None] * HQ
        NSW = 2
        for h in range(NSW):
            ps_q[h] = ps.tile([128, T], F32, tag="ps", name=f"ps_q{h}")
        for j in range(ND):
            st, sp = (j == 0), (j == ND - 1)
            xr = xslice(j)
            js = slice(j * HEAD_DIM, (j + 1) * HEAD_DIM)
            nc.tensor.matmul(ps_k[:], wk_sb[:, js], xr, start=st, stop=sp)
            nc.tensor.matmul(ps_v[:], wv_sb[:, js], xr, start=st, stop=sp)
            for h in range(NSW):
                nc.tensor.matmul(ps_q[h][:], wq_tiles[h][:, js], xr, start=st, stop=sp)

        rope(kT_sb[:], ps_k[:], ck_sb[:], sk_sb[:])
        qts = {}
        for h in range(NSW):
            qts[h] = qtp.tile([128, T], QD, tag="qT", name=f"qT{h}")
            rope(qts[h][:], ps_q[h][:], cq_sb[:], sq_sb[:])

        def q_sweep(h):
            ps_qh = ps.tile([128, T], F32, tag="ps", name=f"ps_q{h}")
            for j in range(ND):
                st, sp = (j == 0), (j == ND - 1)
                js = slice(j * HEAD_DIM, (j + 1) * HEAD_DIM)
                nc.tensor.matmul(
                    ps_qh[:], wq_tiles[h][:, js], xslice(j), start=st, stop=sp
                )
            qt = qtp.tile([128, T], QD, tag="qT", name=f"qT{h}")
            rope(qt[:], ps_qh[:], cq_sb[:], sq_sb[:])
            return qt

        def keep_warm(n=2):
            for _ in range(n):
                nc.tensor.matmul(
                    ps_warm[:], warm_w[:], warm_x[:], start=True, stop=True
                )

        def att_scores(h, qt):
            ps_s = ps.tile([128, T], F32, tag="ps", name=f"ps_s{h}")
            for b in range(BSZ):
                bs = slice(b * 128, (b + 1) * 128)
                nc.tensor.matmul(
                    ps_s[:, bs], qt[:, bs], kT_sb[:, bs], start=True, stop=True
                )
            s_sb = sm.tile([128, T], F32, tag="s", name=f"s{h}")
            nmx = sm.tile([128, BSZ], F32, tag="nmx", name=f"nmx{h}")
            den = sm.tile([128, BSZ], F32, tag="den", name=f"den{h}")
            rden = sm.tile([128, BSZ], F32, tag="rden", name=f"rden{h}")
            p_sb = sm.tile([128, T], BF16, tag="p", name=f"p{h}")
            for b in range(BSZ):
                bs = slice(b * 128, (b + 1) * 128)
                nc.vector.tensor_add(s_sb[:, bs], ps_s[:, bs], mask_sb[:])
                nc.vector.reduce_max(
                    nmx[:, b : b + 1], s_sb[:, bs], axis=AX.X, negate=True
                )
                nc.scalar.activation(
                    p_sb[:, bs],
                    s_sb[:, bs],
                    ACTF.Exp,
                    bias=nmx[:, b : b + 1],
                    accum_out=den[:, b : b + 1],
                )
            nc.vector.reciprocal(rden[:], den[:])
            for b in range(BSZ):
                bs = slice(b * 128, (b + 1) * 128)
                nc.vector.tensor_scalar_mul(p_sb[:, bs], p_sb[:, bs], rden[:, b : b + 1])
            return p_sb

        def att_pv(h, p_sb):
            ps_pt = ps.tile([128, T], BF16, tag="ps", name=f"ps_pt{h}")
            for b in range(BSZ):
                bs = slice(b * 128, (b + 1) * 128)
                nc.tensor.transpose(ps_pt[:, bs], p_sb[:, bs], ident_sb[:])
            pt_sb = sm.tile([128, T], BF16, tag="pt", name=f"pt{h}")
            nc.scalar.copy(pt_sb[:], ps_pt[:])
            ps_o = ps.tile([128, T], F32, tag="ps", name=f"ps_o{h}")
            for b in range(BSZ):
                bs = slice(b * 128, (b + 1) * 128)
                nc.tensor.matmul(
                    ps_o[:, bs], v_sb[:, bs], pt_sb[:, bs], start=True, stop=True
                )
            if h % 2 == 0:
                nc.vector.tensor_copy(oT_sb[:, h * T : (h + 1) * T], ps_o[:])
            else:
                nc.scalar.copy(oT_sb[:, h * T : (h + 1) * T], ps_o[:])

        probs = {}
        qts[2] = q_sweep(2)
        nc.scalar.copy(vT_sb[:], ps_v[:])
        for b in range(BSZ):
            bs = slice(b * 128, (b + 1) * 128)
            ps_t = ps.tile([128, T], BF16, tag="ps")
            nc.tensor.transpose(ps_t[:, 0:128], vT_sb[:, bs], ident_sb[:])
            nc.vector.tensor_copy(v_sb[:, bs], ps_t[:, 0:128])
        probs[0] = att_scores(0, qts[0])
        probs[1] = att_scores(1, qts[1])
        qts[3] = q_sweep(3)
        att_pv(0, probs[0])
        probs[2] = att_scores(2, qts[2])
        att_pv(1, probs[1])
        keep_warm(2)
        probs[3] = att_scores(3, qts[3])
        att_pv(2, probs[2])
        keep_warm(2)
        att_pv(3, probs[3])

        for dt in range(ND):
            ps_y = ps.tile([128, T], F32, tag="ps", name=f"ps_y{dt}")
            for j in range(HQ):
                nc.tensor.matmul(
                    ps_y[:],
                    wo_sb[:, j * DIM + dt * 128 : j * DIM + (dt + 1) * 128],
                    oT_sb[:, j * T : (j + 1) * T],
                    start=(j == 0),
                    stop=(j == HQ - 1),
                )
            y_sb = yp.tile([128, T], BF16, tag="y", name=f"y{dt}")
            if dt % 2 == 0:
                nc.vector.tensor_copy(y_sb[:], ps_y[:])
                nc.sync.dma_start(yT[dt * 128 : (dt + 1) * 128, :], y_sb[:])
            else:
                nc.scalar.copy(y_sb[:], ps_y[:])
                nc.scalar.dma_start(yT[dt * 128 : (dt + 1) * 128, :], y_sb[:])

    nc.compile()
    return nc


def get_nc(fast: bool):
    key = "nc_fast" if fast else "nc_robust"
    if key not in _STATE:
        _STATE[key] = _build_nc_fast() if fast else _build_nc_robust()
    return _STATE[key]


def _prep_in_maps(x, wq, wk, wv, wo, freqs_cos, freqs_sin, mask, fast):
    f32 = np.float32
    bf16 = ml_dtypes.bfloat16
    xd = bf16 if fast else f32
    x = np.asarray(x, f32)
    wq = np.asarray(wq, f32)
    wk = np.asarray(wk, f32)
    wv = np.asarray(wv, f32)
    wo = np.asarray(wo, f32)
    fc = np.asarray(freqs_cos, f32)
    fs = np.asarray(freqs_sin, f32)
    mask = np.asarray(mask, f32)

    # even features first, then odd: (2i, 2i+1) pairs -> (i, i+64)
    perm = np.concatenate([np.arange(0, HEAD_DIM, 2), np.arange(1, HEAD_DIM, 2)])
    wqp = wq.reshape(N_HEADS, HEAD_DIM, DIM)[:, perm, :].reshape(DIM, DIM)
    wkp = wk.reshape(N_KV_HEADS, HEAD_DIM, DIM)[:, perm, :].reshape(
        N_KV_HEADS * HEAD_DIM, DIM
    )

    def sw_x(xmat):  # [T, DIM] -> [128, ND*T]: (p, j*T + t) = x[t, j*128+p]
        return np.ascontiguousarray(
            xmat.T.reshape(ND, 128, T).transpose(1, 0, 2).reshape(128, ND * T)
        )

    def sw_w(wmat):  # [E(128), DIM] -> [128, ND*E]: (p, j*E + e) = w[e, j*128+p]
        E = wmat.shape[0]
        return np.ascontiguousarray(
            wmat.T.reshape(ND, 128, E).transpose(1, 0, 2).reshape(128, ND * E)
        )

    xT = sw_x(x.reshape(T, DIM)).astype(xd)
    C0 = np.vstack([fc.T, fc.T])  # [128, 128]: row p -> cos[t, p % 64]
    S0 = np.vstack([-fs.T, fs.T])
    td = bf16 if fast else f32
    cq = np.tile(C0 * SCALE, (1, BSZ))
    sq = np.tile(S0 * SCALE, (1, BSZ))
    ck = np.tile(C0, (1, BSZ))
    sk = np.tile(S0, (1, BSZ))
    mask1 = np.ascontiguousarray(mask[0, 0])
    ident = np.eye(128, dtype=bf16)

    in_maps = []
    for c in range(NCORES):
        qrows = slice(c * EQ, (c + 1) * EQ)
        krows = slice(c * HEAD_DIM, (c + 1) * HEAD_DIM)
        wq_heads = [
            sw_w(wqp[c * EQ + h * HEAD_DIM : c * EQ + (h + 1) * HEAD_DIM, :])
            for h in range(HQ)
        ]
        # wo: (p, j*DIM + dout) = wo[dout, c*EQ + j*128 + p]
        wo_sw = np.ascontiguousarray(
            wo[:, qrows].T.reshape(HQ, 128, DIM).transpose(1, 0, 2).reshape(128, HQ * DIM)
        ).astype(bf16)
        im = {
            "xT": xT,
            "wqT": np.ascontiguousarray(np.concatenate(wq_heads, axis=1)).astype(xd),
            "wkT": sw_w(wkp[krows, :]).astype(xd),
            "wvT": sw_w(wv[krows, :]).astype(xd),
            "woT": wo_sw,
            "ident": ident,
        }
        if fast:
            im["tabs"] = np.ascontiguousarray(
                np.concatenate([ck, sk, cq, sq], axis=1)
            ).astype(td)
            im["maskb"] = np.ascontiguousarray(np.tile(mask1, (1, BSZ))).astype(bf16)
        else:
            im["mask1"] = mask1
            im["cq"] = np.ascontiguousarray(cq).astype(td)
            im["sq"] = np.ascontiguousarray(sq).astype(td)
            im["ck"] = np.ascontiguousarray(ck).astype(td)
            im["sk"] = np.ascontiguousarray(sk).astype(td)
        in_maps.append(im)
    return in_maps


def _pick_fast(x, wq):
    """bf16 q/k only when softmax logits are smooth (score sigma small).

    score_sigma ~= std(x) * std(wq) * sqrt(DIM * HEAD_DIM) * SCALE. In the
    winner-take-all regime (sigma >> 1) bf16 rounding flips argmaxes, so use
    the fp32r path there.
    """
    sx = float(np.asarray(x, np.float32).std())
    sw = float(np.asarray(wq, np.float32).std())
    sigma = sx * sw * math.sqrt(DIM * HEAD_DIM) * SCALE
    return sigma < 8.0


def kernel(
    x,
    wq,
    wk,
    wv,
    wo,
    cache_k,
    cache_v,
    freqs_cos,
    freqs_sin,
    mask,
    start_pos,
    *,
    trace=False,
    trace_kwargs=None,
):
    global LAST_RESULT
    sp = int(np.asarray(start_pos))
    assert sp == 0, f"kernel specialized for start_pos=0, got {sp}"

    fast = _pick_fast(x, wq)
    in_maps = _prep_in_maps(x, wq, wk, wv, wo, freqs_cos, freqs_sin, mask, fast)
    nc = get_nc(fast)
    res = run_bass_kernel_spmd(
        nc,
        in_maps,
        core_ids=list(range(NCORES)),
        trace=trace,
        **(trace_kwargs or {}),
    )
    LAST_RESULT = res
    if fast:
        acc = np.zeros((128, ND * T), np.float32)
        for c in range(NCORES):
            acc += res.results[c]["yT"].astype(np.float32)
        # unswizzle: y[dt*128+p, t] = yT[p, dt*T+t]
        yfull = acc.reshape(128, ND, T).transpose(1, 0, 2).reshape(DIM, T)
    else:
        yfull = np.zeros((DIM, T), np.float32)
        for c in range(NCORES):
            yfull += res.results[c]["yT"].astype(np.float32)
    return np.ascontiguousarray(yfull.T).reshape(BSZ, SEQLEN, DIM)


# revision 27
# speedup vs baseline: 1.3972x; 1.3972x over previous
"""Tensor-parallel GQA attention prefill for 8 TRN2 NeuronCores.

Sharding: each core owns 4 query heads + 1 kv head (column-shard of
wq/wk/wv by head) and a 512-row slice of wo's input dim (row-shard).
Each core computes a partial output projection over its local heads;
the host sums the 8 partials (equivalent to the all-reduce in the
sharding hint) and transposes back to [b, s, d].

Fast-path schedule, built from the measured baseline trace:
  - One combined PE sweep accumulates K, V, Q0, Q1, Q2 over the 32
    d-tiles; Q3's sweep runs afterwards, interleaved with the RoPE /
    score chains of the earlier heads so the PE never idles (an idle
    PE drops the HAM duty cycle to 4/8 for ~10us -- half-speed
    matmuls afterwards).
  - Inputs stream in consumption order, chunked, on the sync+gpsimd
    DMA rings (the scalar ring starts ~3us late on hw and only
    carries late-needed tensors). The first matmul needs only ~400KB.
  - Softmax: causal mask is preloaded into PSUM by the DVE, scores
    accumulate on top, exp reads PSUM directly with no reduce_max
    (logits are bounded in the smooth regime), so the per-head chain
    is short.
  - The wo projection is chunked per d-tile; the first chunks are
    interleaved between the PV matmuls to cover the softmax chains.
  - y is staged 4 d-tiles at a time and written with large DMAs to a
    pre-swizzled [128, ND*T] output (host unswizzles).

The robust (fp32r) path for winner-take-all softmax inputs keeps the
original proven build.
"""

import math
from contextlib import ExitStack

import ml_dtypes
import numpy as np

import concourse.bass as bass
import concourse.tile as tile
from concourse import bacc, mybir
from concourse.bass_utils import run_bass_kernel_spmd

DIM = 4096
N_HEADS = 32
HEAD_DIM = 128
N_KV_HEADS = 8
BSZ = 4
SEQLEN = 128
T = BSZ * SEQLEN  # 512 tokens
NCORES = 8
HQ = N_HEADS // NCORES  # 4 query heads per core
EQ = HQ * HEAD_DIM  # 512 local q features
ND = DIM // 128  # 32 contraction tiles
SCALE = 1.0 / math.sqrt(HEAD_DIM)

F32 = mybir.dt.float32
F32R = mybir.dt.float32r
BF16 = mybir.dt.bfloat16
AX = mybir.AxisListType
ACTF = mybir.ActivationFunctionType
PSUM = bass.MemorySpace.PSUM

_STATE: dict = {}
LAST_RESULT = None


def _install_ntff_hook():
    """Register the axon NTFF profile hook if the image lacks antenv.axon_hooks.

    Lets run_bass_kernel_spmd(trace=True) return exec_time_ns + perfetto
    under axon. Best-effort: any failure leaves tracing disabled but the
    kernel still runs.
    """
    import os
    import sys
    import types

    try:
        import antenv.axon_hooks  # noqa: F401

        return
    except ImportError:
        pass
    try:
        import antenv
        from trn_agent_boot.trn_boot import _ntff_profile_via_ctypes

        mod = types.ModuleType("antenv.axon_hooks")
        holder = {"hook": None}
        mod.set_axon_ntff_profile_hook = lambda h: holder.__setitem__("hook", h)
        mod.get_axon_ntff_profile_hook = lambda: holder["hook"]
        sys.modules["antenv.axon_hooks"] = mod
        antenv.axon_hooks = mod
        so = "/opt/axon/libaxon_pjrt.so"
        if os.path.exists(so):
            hook = _ntff_profile_via_ctypes(so)
            if hook is not None:
                mod.set_axon_ntff_profile_hook(hook)
    except Exception:
        pass


_install_ntff_hook()

# x-group boundaries (d-tiles): even groups stream on the sync ring,
# odd groups on the gpsimd ring.
XG = [(0, 2), (2, 4), (4, 7), (7, 10), (10, 14), (14, 18), (18, 22),
      (22, 26), (26, 29), (29, 32)]
# weight chunk boundaries (d-tiles), shared by wk/wv/wq0..2
WCH = [(0, 2), (2, 5), (5, 10), (10, 16), (16, 24), (24, 32)]


def _build_nc_fast():
    """Build the bf16 SPMD kernel graph (smooth-softmax regime)."""
    XD = BF16
    QD = BF16
    TD = BF16
    nc = bacc.Bacc(
        "TRN2",
        target_bir_lowering=False,
        debug=False,
        enable_asserts=False,
        num_devices=NCORES,
    )
    xT = nc.dram_tensor("xT", [128, ND * T], XD, kind="ExternalInput").ap()
    wqT = nc.dram_tensor("wqT", [128, HQ * ND * HEAD_DIM], XD, kind="ExternalInput").ap()
    wkT = nc.dram_tensor("wkT", [128, ND * HEAD_DIM], XD, kind="ExternalInput").ap()
    wvT = nc.dram_tensor("wvT", [128, ND * HEAD_DIM], XD, kind="ExternalInput").ap()
    woT = nc.dram_tensor("woT", [128, HQ * DIM], BF16, kind="ExternalInput").ap()
    # causal mask tiled 4x along tokens, bf16 (preloaded into PSUM via PE)
    maskb = nc.dram_tensor("maskb", [128, T], BF16, kind="ExternalInput").ap()
    # tables: ck | sk | cq | sq, each [128, T]
    tabs = nc.dram_tensor("tabs", [128, 4 * T], TD, kind="ExternalInput").ap()
    ident = nc.dram_tensor("ident", [128, 128], BF16, kind="ExternalInput").ap()
    yT = nc.dram_tensor("yT", [128, ND * T], BF16, kind="ExternalOutput").ap()

    with tile.TileContext(nc) as tc, ExitStack() as ctx:
        const = ctx.enter_context(tc.tile_pool(name="const", bufs=1))
        wp = ctx.enter_context(tc.tile_pool(name="wp", bufs=6))
        qtp = ctx.enter_context(tc.tile_pool(name="qtp", bufs=4))
        rt = ctx.enter_context(tc.tile_pool(name="rt", bufs=2))
        sm = ctx.enter_context(tc.tile_pool(name="sm", bufs=3))
        yp = ctx.enter_context(tc.tile_pool(name="yp", bufs=3))
        ps = ctx.enter_context(tc.tile_pool(name="ps", bufs=6, space=PSUM))
        psy = ctx.enter_context(tc.tile_pool(name="psy", bufs=2, space=PSUM))

        # ---- warm tiles (memset on gpsimd before its DMA issues) ----
        warm_w = const.tile([128, 128], BF16, tag="warm_w")
        nc.gpsimd.memset(warm_w[:], 0.0)
        warm_x = const.tile([128, T], BF16, tag="warm_x")
        nc.gpsimd.memset(warm_x[:], 0.0)

        # ---- SBUF tiles fed by DMA ----
        wk_sb = wp.tile([128, ND * HEAD_DIM], XD, tag="w", name="wk")
        wv_sb = wp.tile([128, ND * HEAD_DIM], XD, tag="w", name="wv")
        wq_tiles = [
            wp.tile([128, ND * HEAD_DIM], XD, tag="w", name=f"wq{h}")
            for h in range(3)
        ]
        x_tiles = [None] * len(XG)

        def load_x(gi, eng):
            j0, j1 = XG[gi]
            xg = const.tile([128, (j1 - j0) * T], XD, tag=f"x{gi}", name=f"x{gi}")
            eng.dma_start(xg[:], xT[:, j0 * T : j1 * T])
            x_tiles[gi] = xg

        def load_wch(eng, sb, dram, base, c0, c1):
            # chunk [c0:c1) d-tiles of a [128, ND*HEAD_DIM] weight tile
            eng.dma_start(
                sb[:, c0 * HEAD_DIM : c1 * HEAD_DIM],
                dram[:, base + c0 * HEAD_DIM : base + c1 * HEAD_DIM],
            )

        tabs_sb = const.tile([128, 4 * T], TD, tag="tabs")
        ident_sb = const.tile([128, 128], BF16, tag="ident")
        mask_sb = const.tile([128, T], BF16, tag="mask")
        wq3_sb = wp.tile([128, ND * HEAD_DIM], XD, tag="w", name="wq3")

        # ---- DMA issue order ----
        # sync ring: wk+wq0 chunks and even x groups (consumption order),
        # then half of wo. gpsimd ring: wv+wq1+wq2 chunks, odd x groups,
        # other half of wo, y outputs later. scalar ring: wq3 + tables +
        # ident/mask, gated behind a dummy read of a mid-sweep x group so
        # they transfer only once the critical stream has headroom.
        def sync_w(c):
            c0, c1 = WCH[c]
            load_wch(nc.sync, wk_sb, wkT, 0, c0, c1)
            load_wch(nc.sync, wq_tiles[0], wqT, 0 * DIM, c0, c1)

        def gp_w(c):
            c0, c1 = WCH[c]
            load_wch(nc.gpsimd, wv_sb, wvT, 0, c0, c1)
            load_wch(nc.gpsimd, wq_tiles[1], wqT, 1 * DIM, c0, c1)
            load_wch(nc.gpsimd, wq_tiles[2], wqT, 2 * DIM, c0, c1)

        sync_w(0)
        load_x(0, nc.sync)
        gp_w(0)
        load_x(1, nc.gpsimd)
        sync_w(1)
        load_x(2, nc.sync)
        gp_w(1)
        sync_w(2)
        load_x(4, nc.sync)
        gp_w(2)
        load_x(3, nc.gpsimd)
        sync_w(3)
        gp_w(3)
        load_x(5, nc.gpsimd)
        sync_w(4)
        load_x(6, nc.sync)
        gp_w(4)
        load_x(7, nc.gpsimd)
        sync_w(5)
        load_x(8, nc.sync)
        gp_w(5)
        load_x(9, nc.gpsimd)
        wo_sb = const.tile([128, HQ * DIM], BF16, tag="wo")
        nc.sync.dma_start(wo_sb[:, : 2 * DIM], woT[:, : 2 * DIM])
        nc.gpsimd.dma_start(wo_sb[:, 2 * DIM :], woT[:, 2 * DIM :])
        # scalar ring, gated: wait for xg6 (~mid-sweep) before issuing
        gate = const.tile([128, 1], BF16, tag="gate")
        nc.scalar.copy(gate[:], x_tiles[6][:, 0:1])
        nc.scalar.dma_start(wq3_sb[:, : 16 * HEAD_DIM], wqT[:, 3 * DIM : 3 * DIM + 2048])
        nc.scalar.dma_start(wq3_sb[:, 16 * HEAD_DIM :], wqT[:, 3 * DIM + 2048 : 4 * DIM])
        nc.scalar.dma_start(tabs_sb[:], tabs)
        nc.scalar.dma_start(ident_sb[:], ident)
        nc.scalar.dma_start(mask_sb[:], maskb)

        ck_t = tabs_sb[:, 0:T]
        sk_t = tabs_sb[:, T : 2 * T]
        cq_t = tabs_sb[:, 2 * T : 3 * T]
        sq_t = tabs_sb[:, 3 * T : 4 * T]

        kT_sb = const.tile([128, T], QD, tag="kT")
        vT_sb = const.tile([128, T], BF16, tag="vT")
        v_sb = const.tile([128, BSZ * HEAD_DIM], BF16, tag="v")
        oT_sb = const.tile([128, HQ * T], BF16, tag="oT")

        def xslice(j):
            for gi, (j0, j1) in enumerate(XG):
                if j0 <= j < j1:
                    return x_tiles[gi][:, (j - j0) * T : (j - j0 + 1) * T]
            raise AssertionError(j)

        # ---- PE warm-up: dummy matmuls bridge until the first data
        # lands (~12.5us), lifting the HAM clock gate on the way ----
        ps_warm = ps.tile([128, T], F32, tag="ps", name="warm")
        for _ in range(12):
            nc.tensor.matmul(ps_warm[:], warm_w[:], warm_x[:], start=True, stop=True)
        for _ in range(6):
            nc.tensor.matmul(
                ps_warm[:, 0:128], warm_w[:], warm_x[:, 0:128], start=True, stop=True
            )

        def rope(dst_ap, pssrc, ctab, stab, evict=None):
            # evict once on scalar (bf16), then DVE runs in 16-bit mode
            qe = rt.tile([128, T], BF16, tag="qe")
            nc.scalar.copy(qe[:], pssrc[:])
            swp = rt.tile([128, T], BF16, tag="swp")
            nc.vector.tensor_copy(swp[0:64, :], qe[64:128, :])
            nc.vector.tensor_copy(swp[64:128, :], qe[0:64, :])
            prod = rt.tile([128, T], BF16, tag="prod")
            nc.vector.tensor_mul(prod[:], qe[:], ctab)
            nc.vector.tensor_mul(swp[:], swp[:], stab)
            nc.vector.tensor_add(dst_ap, prod[:], swp[:])

        # ---- combined sweep: K, V, Q0, Q1, Q2 accumulate together over
        # the 32 d-tiles, riding the incoming x stream ----
        ps_k = ps.tile([128, T], F32, tag="ps", name="ps_k")
        ps_v = ps.tile([128, T], F32, tag="ps", name="ps_v")
        ps_q = [None] * HQ
        for h in range(3):
            ps_q[h] = ps.tile([128, T], F32, tag="ps", name=f"ps_q{h}")
        for j in range(ND):
            st, sp = (j == 0), (j == ND - 1)
            xr = xslice(j)
            js = slice(j * HEAD_DIM, (j + 1) * HEAD_DIM)
            nc.tensor.matmul(ps_k[:], wk_sb[:, js], xr, start=st, stop=sp)
            nc.tensor.matmul(ps_v[:], wv_sb[:, js], xr, start=st, stop=sp)
            for h in range(3):
                nc.tensor.matmul(ps_q[h][:], wq_tiles[h][:, js], xr, start=st, stop=sp)

        # rope chains drain PSUM on the DVE; vT eviction on scalar
        rope(kT_sb[:], ps_k[:], ck_t, sk_t)
        nc.scalar.copy(vT_sb[:], ps_v[:])
        qts = {}
        for h in range(3):
            qts[h] = qtp.tile([128, T], QD, tag="qT", name=f"qT{h}")
            rope(qts[h][:], ps_q[h][:], cq_t, sq_t)

        # ---- Q3 sweep, interleaved with attention entry ----
        ps_q3 = ps.tile([128, T], F32, tag="ps", name="ps_q3")

        def q3s(j0, j1):
            for j in range(j0, j1):
                nc.tensor.matmul(
                    ps_q3[:],
                    wq3_sb[:, j * HEAD_DIM : (j + 1) * HEAD_DIM],
                    xslice(j),
                    start=(j == 0),
                    stop=(j == ND - 1),
                )

        def v_transpose():
            for b in range(BSZ):
                bs = slice(b * 128, (b + 1) * 128)
                ps_t = ps.tile([128, T], BF16, tag="ps", name=f"ps_vt{b}")
                nc.tensor.transpose(ps_t[:, 0:128], vT_sb[:, bs], ident_sb[:])
                nc.vector.tensor_copy(v_sb[:, bs], ps_t[:, 0:128])

        s_tiles = {}
        den = sm.tile([128, BSZ * HQ], F32, tag="den")
        rden = sm.tile([128, BSZ * HQ], F32, tag="rden")

        def att_scores(h, qt):
            # mask preloaded into PSUM via identity matmul; scores
            # accumulate on top (start=False)
            ps_s = ps.tile([128, T], F32, tag="ps", name=f"ps_s{h}")
            nc.tensor.matmul(ps_s[:], ident_sb[:], mask_sb[:], start=True, stop=False)
            for b in range(BSZ):
                bs = slice(b * 128, (b + 1) * 128)
                nc.tensor.matmul(
                    ps_s[:, bs], qt[:, bs], kT_sb[:, bs], start=False, stop=True
                )
            s_tiles[h] = ps_s

        def att_soft(h):
            # no reduce_max: logits are bounded in the smooth regime.
            # single exp instruction per head; den via reduce_sum on the
            # idle gpsimd engine.
            ps_s = s_tiles[h]
            hh = slice(h * BSZ, (h + 1) * BSZ)
            p_sb = sm.tile([128, T], BF16, tag="p", name=f"p{h}")
            nc.scalar.activation(p_sb[:], ps_s[:], ACTF.Exp)
            for b in range(BSZ):
                bs = slice(b * 128, (b + 1) * 128)
                nc.vector.reduce_sum(
                    den[:, h * BSZ + b : h * BSZ + b + 1], p_sb[:, bs], axis=AX.X
                )
            nc.vector.reciprocal(rden[:, hh], den[:, hh])
            for b in range(BSZ):
                bs = slice(b * 128, (b + 1) * 128)
                nc.vector.tensor_scalar_mul(
                    p_sb[:, bs], p_sb[:, bs], rden[:, h * BSZ + b : h * BSZ + b + 1]
                )
            return p_sb

        pts = {}

        def att_ptrans(h, p_sb):
            ps_pt = ps.tile([128, T], BF16, tag="ps", name=f"ps_pt{h}")
            for b in range(BSZ):
                bs = slice(b * 128, (b + 1) * 128)
                nc.tensor.transpose(ps_pt[:, bs], p_sb[:, bs], ident_sb[:])
            pt_sb = sm.tile([128, T], BF16, tag="pt", name=f"pt{h}")
            if h % 2 == 0:
                nc.vector.tensor_copy(pt_sb[:], ps_pt[:])
            else:
                nc.scalar.copy(pt_sb[:], ps_pt[:])
            pts[h] = pt_sb

        def att_pv(h):
            ps_o = ps.tile([128, T], F32, tag="ps", name=f"ps_o{h}")
            for b in range(BSZ):
                bs = slice(b * 128, (b + 1) * 128)
                nc.tensor.matmul(
                    ps_o[:, bs], v_sb[:, bs], pts[h][:, bs], start=True, stop=True
                )
            nc.scalar.copy(oT_sb[:, h * T : (h + 1) * T], ps_o[:])

        # ---- wo projection, chunked per d-tile ----
        ps_ys = {}

        def wo_mm(dt, j):
            if j == 0:
                # dt>=8: rotate through the main ps pool too (its att
                # tiles are gone by then), deepening the drain pipeline
                pool = psy if dt < 8 or dt % 2 == 0 else ps
                tg = "psy" if pool is psy else "ps"
                ps_ys[dt] = pool.tile([128, T], F32, tag=tg, name=f"ps_y{dt}")
            nc.tensor.matmul(
                ps_ys[dt][:],
                wo_sb[:, j * DIM + dt * 128 : j * DIM + (dt + 1) * 128],
                oT_sb[:, j * T : (j + 1) * T],
                start=(j == 0),
                stop=(j == HQ - 1),
            )

        # y staging: groups of 4 d-tiles (last two groups of 2)
        YG = [(0, 4), (4, 8), (8, 12), (12, 16), (16, 20), (20, 24),
              (24, 28), (28, 30), (30, 32)]
        ystage = {}

        def y_drain(dt):
            for g0, g1 in YG:
                if g0 <= dt < g1:
                    break
            if dt == g0:
                ystage[g0] = yp.tile([128, (g1 - g0) * T], BF16, tag="y", name=f"y{g0}")
            # split the drain across vector+scalar so its latency stays
            # below the 4-matmul chunk time (psy ping-pong never stalls)
            dst = ystage[g0][:, (dt - g0) * T : (dt - g0 + 1) * T]
            H = T // 2
            nc.vector.tensor_copy(dst[:, 0:H], ps_ys[dt][:, 0:H])
            nc.scalar.copy(dst[:, H:T], ps_ys[dt][:, H:T])
            del ps_ys[dt]
            if dt == g1 - 1:
                nc.sync.dma_start(yT[:, g0 * T : g1 * T], ystage[g0][:])

        # ---- attention entry, interleaved with the q3 sweep ----
        q3s(0, 8)
        v_transpose()
        att_scores(0, qts[0])
        p0 = att_soft(0)
        q3s(8, 16)
        att_scores(1, qts[1])
        p1 = att_soft(1)
        q3s(16, 24)
        att_scores(2, qts[2])
        p2 = att_soft(2)
        q3s(24, ND)
        qts[3] = qtp.tile([128, T], QD, tag="qT", name="qT3")
        rope(qts[3][:], ps_q3[:], cq_t, sq_t, evict="vector")
        att_ptrans(0, p0)
        att_pv(0)
        att_ptrans(1, p1)
        att_pv(1)
        att_scores(3, qts[3])
        p3 = att_soft(3)
        wo_mm(0, 0)
        wo_mm(1, 0)
        wo_mm(0, 1)
        wo_mm(1, 1)
        att_ptrans(2, p2)
        att_pv(2)
        wo_mm(0, 2)
        wo_mm(1, 2)
        att_ptrans(3, p3)
        att_pv(3)
        wo_mm(0, 3)
        wo_mm(1, 3)
        y_drain(0)
        y_drain(1)
        for dt in range(2, ND):
            for j in range(HQ):
                wo_mm(dt, j)
            y_drain(dt)

    nc.compile()
    return nc


def _build_nc_robust():
    """Original proven build for the fp32r (winner-take-all) path."""
    XD = F32R
    QD = F32
    TD = F32
    nc = bacc.Bacc(
        "TRN2",
        target_bir_lowering=False,
        debug=False,
        enable_asserts=False,
        num_devices=NCORES,
    )
    xT = nc.dram_tensor("xT", [128, ND * T], XD, kind="ExternalInput").ap()
    wqT = nc.dram_tensor("wqT", [128, HQ * ND * HEAD_DIM], XD, kind="ExternalInput").ap()
    wkT = nc.dram_tensor("wkT", [128, ND * HEAD_DIM], XD, kind="ExternalInput").ap()
    wvT = nc.dram_tensor("wvT", [128, ND * HEAD_DIM], XD, kind="ExternalInput").ap()
    woT = nc.dram_tensor("woT", [128, HQ * DIM], BF16, kind="ExternalInput").ap()
    mask1 = nc.dram_tensor("mask1", [128, 128], F32, kind="ExternalInput").ap()
    cq = nc.dram_tensor("cq", [128, T], TD, kind="ExternalInput").ap()
    sq = nc.dram_tensor("sq", [128, T], TD, kind="ExternalInput").ap()
    ck = nc.dram_tensor("ck", [128, T], TD, kind="ExternalInput").ap()
    sk = nc.dram_tensor("sk", [128, T], TD, kind="ExternalInput").ap()
    ident = nc.dram_tensor("ident", [128, 128], BF16, kind="ExternalInput").ap()
    yT = nc.dram_tensor("yT", [DIM, T], BF16, kind="ExternalOutput").ap()

    with tile.TileContext(nc) as tc, ExitStack() as ctx:
        const = ctx.enter_context(tc.tile_pool(name="const", bufs=1))
        wp = ctx.enter_context(tc.tile_pool(name="wp", bufs=4))
        qtp = ctx.enter_context(tc.tile_pool(name="qtp", bufs=4))
        rt = ctx.enter_context(tc.tile_pool(name="rt", bufs=1))
        sm = ctx.enter_context(tc.tile_pool(name="sm", bufs=2))
        yp = ctx.enter_context(tc.tile_pool(name="yp", bufs=2))
        ps = ctx.enter_context(tc.tile_pool(name="ps", bufs=7, space=PSUM))
        wps = ctx.enter_context(tc.tile_pool(name="wps", bufs=1, space=PSUM))

        warm_w = const.tile([128, 128], BF16, tag="warm_w")
        nc.gpsimd.memset(warm_w[:], 0.0)
        warm_x = const.tile([128, T], BF16, tag="warm_x")
        nc.gpsimd.memset(warm_x[:], 0.0)
        ps_warm = wps.tile([128, T], F32, tag="wps")
        for _ in range(10):
            nc.tensor.matmul(ps_warm[:], warm_w[:], warm_x[:], start=True, stop=True)

        wk_sb = wp.tile([128, ND * HEAD_DIM], XD, tag="w", name="wk")
        nc.sync.dma_start(wk_sb[:], wkT)
        wv_sb = wp.tile([128, ND * HEAD_DIM], XD, tag="w", name="wv")
        nc.scalar.dma_start(wv_sb[:], wvT)

        XGROUPS = [2, 2, 2, 2, 4, 4, 4, 4, 4, 4]
        x_tiles = [None] * len(XGROUPS)
        xg_col = []
        j0 = 0
        for gi, gd in enumerate(XGROUPS):
            xg_col.append((j0, gd))
            j0 += gd

        def load_x(gi, eng):
            j0, gd = xg_col[gi]
            xg = const.tile([128, gd * T], XD, tag=f"x{gi}", name=f"x{gi}")
            eng.dma_start(xg[:], xT[:, j0 * T : (j0 + gd) * T])
            x_tiles[gi] = xg

        wq_tiles = [None] * HQ

        def load_wq(h, eng):
            wqt = wp.tile([128, ND * HEAD_DIM], XD, tag="w", name=f"wq{h}")
            eng.dma_start(wqt[:], wqT[:, h * DIM : (h + 1) * DIM])
            wq_tiles[h] = wqt

        load_x(0, nc.sync)
        load_wq(0, nc.scalar)
        load_wq(1, nc.sync)
        load_x(1, nc.scalar)
        load_wq(2, nc.sync)
        load_x(2, nc.scalar)
        load_wq(3, nc.sync)
        ident_sb = const.tile([128, 128], BF16, tag="ident")
        nc.scalar.dma_start(ident_sb[:], ident)
        ck_sb = const.tile([128, T], TD, tag="ck")
        nc.scalar.dma_start(ck_sb[:], ck)
        sk_sb = const.tile([128, T], TD, tag="sk")
        nc.scalar.dma_start(sk_sb[:], sk)
        cq_sb = const.tile([128, T], TD, tag="cq")
        nc.scalar.dma_start(cq_sb[:], cq)
        sq_sb = const.tile([128, T], TD, tag="sq")
        nc.scalar.dma_start(sq_sb[:], sq)
        mask_sb = const.tile([128, 128], F32, tag="mask")
        nc.scalar.dma_start(mask_sb[:], mask1)
        for gi in range(3, len(XGROUPS)):
            load_x(gi, nc.scalar if gi % 2 == 0 else nc.sync)
        wo_sb = const.tile([128, HQ * DIM], BF16, tag="wo")
        nc.sync.dma_start(wo_sb[:, : 2 * DIM], woT[:, : 2 * DIM])
        nc.scalar.dma_start(wo_sb[:, 2 * DIM :], woT[:, 2 * DIM :])

        kT_sb = const.tile([128, T], QD, tag="kT")
        vT_sb = const.tile([128, T], BF16, tag="vT")
        v_sb = const.tile([128, BSZ * HEAD_DIM], BF16, tag="v")
        oT_sb = const.tile([128, HQ * T], BF16, tag="oT")

        def xslice(j):
            gi = 0
            j0 = 0
            for i, (jj0, gd) in enumerate(xg_col):
                if jj0 <= j < jj0 + gd:
                    gi, j0 = i, jj0
                    break
            return x_tiles[gi][:, (j - j0) * T : (j - j0 + 1) * T]

        def rope(dst_ap, pssrc, ctab, stab):
            swp = rt.tile([128, T], F32, tag="swp")
            nc.scalar.copy(swp[0:64, :], pssrc[64:128, :])
            nc.scalar.copy(swp[64:128, :], pssrc[0:64, :])
            prod = rt.tile([128, T], F32, tag="prod")
            nc.vector.tensor_mul(prod[:], pssrc[:], ctab)
            nc.vector.tensor_mul(swp[:], swp[:], stab)
            nc.vector.tensor_add(dst_ap, prod[:], swp[:])

        ps_k = ps.tile([128, T], F32, tag="ps")
        ps_v = ps.tile([128, T], F32, tag="ps")
        ps_q = [None] * HQ
        NSW = 2
        for h in range(NSW):
            ps_q[h] = ps.tile([128, T], F32, tag="ps", name=f"ps_q{h}")
        for j in range(ND):
            st, sp = (j == 0), (j == ND - 1)
            xr = xslice(j)
            js = slice(j * HEAD_DIM, (j + 1) * HEAD_DIM)
            nc.tensor.matmul(ps_k[:], wk_sb[:, js], xr, start=st, stop=sp)
            nc.tensor.matmul(ps_v[:], wv_sb[:, js], xr, start=st, stop=sp)
            for h in range(NSW):
                nc.tensor.matmul(ps_q[h][:], wq_tiles[h][:, js], xr, start=st, stop=sp)

        rope(kT_sb[:], ps_k[:], ck_sb[:], sk_sb[:])
        qts = {}
        for h in range(NSW):
            qts[h] = qtp.tile([128, T], QD, tag="qT", name=f"qT{h}")
            rope(qts[h][:], ps_q[h][:], cq_sb[:], sq_sb[:])

        def q_sweep(h):
            ps_qh = ps.tile([128, T], F32, tag="ps", name=f"ps_q{h}")
            for j in range(ND):
                st, sp = (j == 0), (j == ND - 1)
                js = slice(j * HEAD_DIM, (j + 1) * HEAD_DIM)
                nc.tensor.matmul(
                    ps_qh[:], wq_tiles[h][:, js], xslice(j), start=st, stop=sp
                )
            qt = qtp.tile([128, T], QD, tag="qT", name=f"qT{h}")
            rope(qt[:], ps_qh[:], cq_sb[:], sq_sb[:])
            return qt

        def keep_warm(n=2):
            for _ in range(n):
                nc.tensor.matmul(
                    ps_warm[:], warm_w[:], warm_x[:], start=True, stop=True
                )

        def att_scores(h, qt):
            ps_s = ps.tile([128, T], F32, tag="ps", name=f"ps_s{h}")
            for b in range(BSZ):
                bs = slice(b * 128, (b + 1) * 128)
                nc.tensor.matmul(
                    ps_s[:, bs], qt[:, bs], kT_sb[:, bs], start=True, stop=True
                )
            s_sb = sm.tile([128, T], F32, tag="s", name=f"s{h}")
            nmx = sm.tile([128, BSZ], F32, tag="nmx", name=f"nmx{h}")
            den = sm.tile([128, BSZ], F32, tag="den", name=f"den{h}")
            rden = sm.tile([128, BSZ], F32, tag="rden", name=f"rden{h}")
            p_sb = sm.tile([128, T], BF16, tag="p", name=f"p{h}")
            for b in range(BSZ):
                bs = slice(b * 128, (b + 1) * 128)
                nc.vector.tensor_add(s_sb[:, bs], ps_s[:, bs], mask_sb[:])
                nc.vector.reduce_max(
                    nmx[:, b : b + 1], s_sb[:, bs], axis=AX.X, negate=True
                )
                nc.scalar.activation(
                    p_sb[:, bs],
                    s_sb[:, bs],
                    ACTF.Exp,
                    bias=nmx[:, b : b + 1],
                    accum_out=den[:, b : b + 1],
                )
            nc.vector.reciprocal(rden[:], den[:])
            for b in range(BSZ):
                bs = slice(b * 128, (b + 1) * 128)
                nc.vector.tensor_scalar_mul(p_sb[:, bs], p_sb[:, bs], rden[:, b : b + 1])
            return p_sb

        def att_pv(h, p_sb):
            ps_pt = ps.tile([128, T], BF16, tag="ps", name=f"ps_pt{h}")
            for b in range(BSZ):
                bs = slice(b * 128, (b + 1) * 128)
                nc.tensor.transpose(ps_pt[:, bs], p_sb[:, bs], ident_sb[:])
            pt_sb = sm.tile([128, T], BF16, tag="pt", name=f"pt{h}")
            nc.scalar.copy(pt_sb[:], ps_pt[:])
            ps_o = ps.tile([128, T], F32, tag="ps", name=f"ps_o{h}")
            for b in range(BSZ):
                bs = slice(b * 128, (b + 1) * 128)
                nc.tensor.matmul(
                    ps_o[:, bs], v_sb[:, bs], pt_sb[:, bs], start=True, stop=True
                )
            if h % 2 == 0:
                nc.vector.tensor_copy(oT_sb[:, h * T : (h + 1) * T], ps_o[:])
            else:
                nc.scalar.copy(oT_sb[:, h * T : (h + 1) * T], ps_o[:])

        probs = {}
        qts[2] = q_sweep(2)
        nc.scalar.copy(vT_sb[:], ps_v[:])
        for b in range(BSZ):
            bs = slice(b * 128, (b + 1) * 128)
            ps_t = ps.tile([128, T], BF16, tag="ps")
            nc.tensor.transpose(ps_t[:, 0:128], vT_sb[:, bs], ident_sb[:])
            nc.vector.tensor_copy(v_sb[:, bs], ps_t[:, 0:128])
        probs[0] = att_scores(0, qts[0])
        probs[1] = att_scores(1, qts[1])
        qts[3] = q_sweep(3)
        att_pv(0, probs[0])
        probs[2] = att_scores(2, qts[2])
        att_pv(1, probs[1])
        keep_warm(2)
        probs[3] = att_scores(3, qts[3])
        att_pv(2, probs[2])
        keep_warm(2)
        att_pv(3, probs[3])

        for dt in range(ND):
            ps_y = ps.tile([128, T], F32, tag="ps", name=f"ps_y{dt}")
            for j in range(HQ):
                nc.tensor.matmul(
                    ps_y[:],
                    wo_sb[:, j * DIM + dt * 128 : j * DIM + (dt + 1) * 128],
                    oT_sb[:, j * T : (j + 1) * T],
                    start=(j == 0),
                    stop=(j == HQ - 1),
                )
            y_sb = yp.tile([128, T], BF16, tag="y", name=f"y{dt}")
            if dt % 2 == 0:
                nc.vector.tensor_copy(y_sb[:], ps_y[:])
                nc.sync.dma_start(yT[dt * 128 : (dt + 1) * 128, :], y_sb[:])
            else:
                nc.scalar.copy(y_sb[:], ps_y[:])
                nc.scalar.dma_start(yT[dt * 128 : (dt + 1) * 128, :], y_sb[:])

    nc.compile()
    return nc


def get_nc(fast: bool):
    key = "nc_fast" if fast else "nc_robust"
    if key not in _STATE:
        _STATE[key] = _build_nc_fast() if fast else _build_nc_robust()
    return _STATE[key]


def _prep_in_maps(x, wq, wk, wv, wo, freqs_cos, freqs_sin, mask, fast):
    f32 = np.float32
    bf16 = ml_dtypes.bfloat16
    xd = bf16 if fast else f32
    x = np.asarray(x, f32)
    wq = np.asarray(wq, f32)
    wk = np.asarray(wk, f32)
    wv = np.asarray(wv, f32)
    wo = np.asarray(wo, f32)
    fc = np.asarray(freqs_cos, f32)
    fs = np.asarray(freqs_sin, f32)
    mask = np.asarray(mask, f32)

    # even features first, then odd: (2i, 2i+1) pairs -> (i, i+64)
    perm = np.concatenate([np.arange(0, HEAD_DIM, 2), np.arange(1, HEAD_DIM, 2)])
    wqp = wq.reshape(N_HEADS, HEAD_DIM, DIM)[:, perm, :].reshape(DIM, DIM)
    wkp = wk.reshape(N_KV_HEADS, HEAD_DIM, DIM)[:, perm, :].reshape(
        N_KV_HEADS * HEAD_DIM, DIM
    )

    def sw_x(xmat):  # [T, DIM] -> [128, ND*T]: (p, j*T + t) = x[t, j*128+p]
        return np.ascontiguousarray(
            xmat.T.reshape(ND, 128, T).transpose(1, 0, 2).reshape(128, ND * T)
        )

    def sw_w(wmat):  # [E(128), DIM] -> [128, ND*E]: (p, j*E + e) = w[e, j*128+p]
        E = wmat.shape[0]
        return np.ascontiguousarray(
            wmat.T.reshape(ND, 128, E).transpose(1, 0, 2).reshape(128, ND * E)
        )

    xT = sw_x(x.reshape(T, DIM)).astype(xd)
    C0 = np.vstack([fc.T, fc.T])  # [128, 128]: row p -> cos[t, p % 64]
    S0 = np.vstack([-fs.T, fs.T])
    td = bf16 if fast else f32
    cq = np.tile(C0 * SCALE, (1, BSZ))
    sq = np.tile(S0 * SCALE, (1, BSZ))
    ck = np.tile(C0, (1, BSZ))
    sk = np.tile(S0, (1, BSZ))
    mask1 = np.ascontiguousarray(mask[0, 0])
    ident = np.eye(128, dtype=bf16)

    in_maps = []
    for c in range(NCORES):
        qrows = slice(c * EQ, (c + 1) * EQ)
        krows = slice(c * HEAD_DIM, (c + 1) * HEAD_DIM)
        wq_heads = [
            sw_w(wqp[c * EQ + h * HEAD_DIM : c * EQ + (h + 1) * HEAD_DIM, :])
            for h in range(HQ)
        ]
        # wo: (p, j*DIM + dout) = wo[dout, c*EQ + j*128 + p]
        wo_sw = np.ascontiguousarray(
            wo[:, qrows].T.reshape(HQ, 128, DIM).transpose(1, 0, 2).reshape(128, HQ * DIM)
        ).astype(bf16)
        im = {
            "xT": xT,
            "wqT": np.ascontiguousarray(np.concatenate(wq_heads, axis=1)).astype(xd),
            "wkT": sw_w(wkp[krows, :]).astype(xd),
            "wvT": sw_w(wv[krows, :]).astype(xd),
            "woT": wo_sw,
            "ident": ident,
        }
        if fast:
            im["tabs"] = np.ascontiguousarray(
                np.concatenate([ck, sk, cq, sq], axis=1)
            ).astype(td)
            im["maskb"] = np.ascontiguousarray(np.tile(mask1, (1, BSZ))).astype(bf16)
        else:
            im["mask1"] = mask1
            im["cq"] = np.ascontiguousarray(cq).astype(td)
            im["sq"] = np.ascontiguousarray(sq).astype(td)
            im["ck"] = np.ascontiguousarray(ck).astype(td)
            im["sk"] = np.ascontiguousarray(sk).astype(td)
        in_maps.append(im)
    return in_maps


def _pick_fast(x, wq):
    """bf16 q/k only when softmax logits are smooth (score sigma small).

    score_sigma ~= std(x) * std(wq) * sqrt(DIM * HEAD_DIM) * SCALE. In the
    winner-take-all regime (sigma >> 1) bf16 rounding flips argmaxes, so use
    the fp32r path there.
    """
    sx = float(np.asarray(x, np.float32).std())
    sw = float(np.asarray(wq, np.float32).std())
    sigma = sx * sw * math.sqrt(DIM * HEAD_DIM) * SCALE
    return sigma < 8.0


def kernel(
    x,
    wq,
    wk,
    wv,
    wo,
    cache_k,
    cache_v,
    freqs_cos,
    freqs_sin,
    mask,
    start_pos,
    *,
    trace=False,
    trace_kwargs=None,
):
    global LAST_RESULT
    sp = int(np.asarray(start_pos))
    assert sp == 0, f"kernel specialized for start_pos=0, got {sp}"

    fast = _pick_fast(x, wq)
    in_maps = _prep_in_maps(x, wq, wk, wv, wo, freqs_cos, freqs_sin, mask, fast)
    nc = get_nc(fast)
    res = run_bass_kernel_spmd(
        nc,
        in_maps,
        core_ids=list(range(NCORES)),
        trace=trace,
        **(trace_kwargs or {}),
    )
    LAST_RESULT = res
    if fast:
        acc = np.zeros((128, ND * T), np.float32)
        for c in range(NCORES):
            acc += res.results[c]["yT"].astype(np.float32)
        # unswizzle: y[dt*128+p, t] = yT[p, dt*T+t]
        yfull = acc.reshape(128, ND, T).transpose(1, 0, 2).reshape(DIM, T)
    else:
        yfull = np.zeros((DIM, T), np.float32)
        for c in range(NCORES):
            yfull += res.results[c]["yT"].astype(np.float32)
    return np.ascontiguousarray(yfull.T).reshape(BSZ, SEQLEN, DIM)


# revision 33
# speedup vs baseline: 1.4010x; 1.0027x over previous
"""Tensor-parallel GQA attention prefill for 8 TRN2 NeuronCores.

Sharding: each core owns 4 query heads + 1 kv head (column-shard of
wq/wk/wv by head) and a 512-row slice of wo's input dim (row-shard).
Each core computes a partial output projection over its local heads;
the host sums the 8 partials (equivalent to the all-reduce in the
sharding hint) and transposes back to [b, s, d].

Fast-path schedule, built from the measured baseline trace:
  - One combined PE sweep accumulates K, V, Q0, Q1, Q2 over the 32
    d-tiles; Q3's sweep runs afterwards, interleaved with the RoPE /
    score chains of the earlier heads so the PE never idles (an idle
    PE drops the HAM duty cycle to 4/8 for ~10us -- half-speed
    matmuls afterwards).
  - Inputs stream in consumption order, chunked, on the sync+gpsimd
    DMA rings (the scalar ring starts ~3us late on hw and only
    carries late-needed tensors). The first matmul needs only ~400KB.
  - Softmax: causal mask is preloaded into PSUM by the DVE, scores
    accumulate on top, exp reads PSUM directly with no reduce_max
    (logits are bounded in the smooth regime), so the per-head chain
    is short.
  - The wo projection is chunked per d-tile; the first chunks are
    interleaved between the PV matmuls to cover the softmax chains.
  - y is staged 4 d-tiles at a time and written with large DMAs to a
    pre-swizzled [128, ND*T] output (host unswizzles).

The robust (fp32r) path for winner-take-all softmax inputs keeps the
original proven build.
"""

import math
from contextlib import ExitStack

import ml_dtypes
import numpy as np

import concourse.bass as bass
import concourse.tile as tile
from concourse import bacc, mybir
from concourse.bass_utils import run_bass_kernel_spmd

DIM = 4096
N_HEADS = 32
HEAD_DIM = 128
N_KV_HEADS = 8
BSZ = 4
SEQLEN = 128
T = BSZ * SEQLEN  # 512 tokens
NCORES = 8
HQ = N_HEADS // NCORES  # 4 query heads per core
EQ = HQ * HEAD_DIM  # 512 local q features
ND = DIM // 128  # 32 contraction tiles
SCALE = 1.0 / math.sqrt(HEAD_DIM)

F32 = mybir.dt.float32
F32R = mybir.dt.float32r
BF16 = mybir.dt.bfloat16
AX = mybir.AxisListType
ACTF = mybir.ActivationFunctionType
PSUM = bass.MemorySpace.PSUM

_STATE: dict = {}
LAST_RESULT = None


def _install_ntff_hook():
    """Register the axon NTFF profile hook if the image lacks antenv.axon_hooks.

    Lets run_bass_kernel_spmd(trace=True) return exec_time_ns + perfetto
    under axon. Best-effort: any failure leaves tracing disabled but the
    kernel still runs.
    """
    import os
    import sys
    import types

    try:
        import antenv.axon_hooks  # noqa: F401

        return
    except ImportError:
        pass
    try:
        import antenv
        from trn_agent_boot.trn_boot import _ntff_profile_via_ctypes

        mod = types.ModuleType("antenv.axon_hooks")
        holder = {"hook": None}
        mod.set_axon_ntff_profile_hook = lambda h: holder.__setitem__("hook", h)
        mod.get_axon_ntff_profile_hook = lambda: holder["hook"]
        sys.modules["antenv.axon_hooks"] = mod
        antenv.axon_hooks = mod
        so = "/opt/axon/libaxon_pjrt.so"
        if os.path.exists(so):
            hook = _ntff_profile_via_ctypes(so)
            if hook is not None:
                mod.set_axon_ntff_profile_hook(hook)
    except Exception:
        pass


_install_ntff_hook()

# x-group boundaries (d-tiles): even groups stream on the sync ring,
# odd groups on the gpsimd ring.
XG = [(0, 2), (2, 4), (4, 7), (7, 10), (10, 14), (14, 18), (18, 22),
      (22, 26), (26, 29), (29, 32)]
# weight chunk boundaries (d-tiles), shared by wk/wv/wq0..2
WCH = [(0, 2), (2, 5), (5, 10), (10, 16), (16, 24), (24, 32)]


def _build_nc_fast():
    """Build the bf16 SPMD kernel graph (smooth-softmax regime)."""
    XD = BF16
    QD = BF16
    TD = BF16
    nc = bacc.Bacc(
        "TRN2",
        target_bir_lowering=False,
        debug=False,
        enable_asserts=False,
        num_devices=NCORES,
    )
    xT = nc.dram_tensor("xT", [128, ND * T], XD, kind="ExternalInput").ap()
    wqT = nc.dram_tensor("wqT", [128, HQ * ND * HEAD_DIM], XD, kind="ExternalInput").ap()
    wkT = nc.dram_tensor("wkT", [128, ND * HEAD_DIM], XD, kind="ExternalInput").ap()
    wvT = nc.dram_tensor("wvT", [128, ND * HEAD_DIM], XD, kind="ExternalInput").ap()
    woT = nc.dram_tensor("woT", [128, HQ * DIM], BF16, kind="ExternalInput").ap()
    # causal mask tiled 4x along tokens, bf16 (preloaded into PSUM via PE)
    maskb = nc.dram_tensor("maskb", [128, T], BF16, kind="ExternalInput").ap()
    # tables: ck | sk | cq | sq, each [128, T]
    tabs = nc.dram_tensor("tabs", [128, 4 * T], TD, kind="ExternalInput").ap()
    ident = nc.dram_tensor("ident", [128, 128], BF16, kind="ExternalInput").ap()
    yT = nc.dram_tensor("yT", [128, ND * T], BF16, kind="ExternalOutput").ap()

    with tile.TileContext(nc) as tc, ExitStack() as ctx:
        const = ctx.enter_context(tc.tile_pool(name="const", bufs=1))
        wp = ctx.enter_context(tc.tile_pool(name="wp", bufs=6))
        qtp = ctx.enter_context(tc.tile_pool(name="qtp", bufs=4))
        rt = ctx.enter_context(tc.tile_pool(name="rt", bufs=3))
        sm = ctx.enter_context(tc.tile_pool(name="sm", bufs=3))
        yp = ctx.enter_context(tc.tile_pool(name="yp", bufs=3))
        ps = ctx.enter_context(tc.tile_pool(name="ps", bufs=6, space=PSUM))
        psy = ctx.enter_context(tc.tile_pool(name="psy", bufs=2, space=PSUM))

        # ---- warm tiles (memset on gpsimd before its DMA issues) ----
        warm_w = const.tile([128, 128], BF16, tag="warm_w")
        nc.gpsimd.memset(warm_w[:], 0.0)
        warm_x = const.tile([128, T], BF16, tag="warm_x")
        nc.gpsimd.memset(warm_x[:], 0.0)

        # ---- SBUF tiles fed by DMA ----
        wk_sb = wp.tile([128, ND * HEAD_DIM], XD, tag="w", name="wk")
        wv_sb = wp.tile([128, ND * HEAD_DIM], XD, tag="w", name="wv")
        wq_tiles = [
            wp.tile([128, ND * HEAD_DIM], XD, tag="w", name=f"wq{h}")
            for h in range(3)
        ]
        x_tiles = [None] * len(XG)

        def load_x(gi, eng):
            j0, j1 = XG[gi]
            xg = const.tile([128, (j1 - j0) * T], XD, tag=f"x{gi}", name=f"x{gi}")
            eng.dma_start(xg[:], xT[:, j0 * T : j1 * T])
            x_tiles[gi] = xg

        def load_wch(eng, sb, dram, base, c0, c1):
            # chunk [c0:c1) d-tiles of a [128, ND*HEAD_DIM] weight tile
            eng.dma_start(
                sb[:, c0 * HEAD_DIM : c1 * HEAD_DIM],
                dram[:, base + c0 * HEAD_DIM : base + c1 * HEAD_DIM],
            )

        tabs_sb = const.tile([128, 4 * T], TD, tag="tabs")
        ident_sb = const.tile([128, 128], BF16, tag="ident")
        mask_sb = const.tile([128, T], BF16, tag="mask")
        wq3_sb = wp.tile([128, ND * HEAD_DIM], XD, tag="w", name="wq3")

        # ---- DMA issue order ----
        # sync ring: wk+wq0 chunks and even x groups (consumption order),
        # then half of wo. gpsimd ring: wv+wq1+wq2 chunks, odd x groups,
        # other half of wo, y outputs later. scalar ring: wq3 + tables +
        # ident/mask, gated behind a dummy read of a mid-sweep x group so
        # they transfer only once the critical stream has headroom.
        def sync_w(c):
            c0, c1 = WCH[c]
            load_wch(nc.sync, wk_sb, wkT, 0, c0, c1)
            load_wch(nc.sync, wq_tiles[0], wqT, 0 * DIM, c0, c1)

        def gp_w(c):
            c0, c1 = WCH[c]
            load_wch(nc.gpsimd, wv_sb, wvT, 0, c0, c1)
            load_wch(nc.gpsimd, wq_tiles[1], wqT, 1 * DIM, c0, c1)
            load_wch(nc.gpsimd, wq_tiles[2], wqT, 2 * DIM, c0, c1)

        sync_w(0)
        load_x(0, nc.sync)
        gp_w(0)
        load_x(1, nc.gpsimd)
        sync_w(1)
        load_x(2, nc.sync)
        gp_w(1)
        sync_w(2)
        load_x(4, nc.sync)
        gp_w(2)
        load_x(3, nc.gpsimd)
        sync_w(3)
        gp_w(3)
        load_x(5, nc.gpsimd)
        sync_w(4)
        load_x(6, nc.sync)
        gp_w(4)
        load_x(7, nc.gpsimd)
        sync_w(5)
        load_x(8, nc.sync)
        gp_w(5)
        load_x(9, nc.gpsimd)
        wo_sb = const.tile([128, HQ * DIM], BF16, tag="wo")
        nc.sync.dma_start(wo_sb[:, : 2 * DIM], woT[:, : 2 * DIM])
        nc.gpsimd.dma_start(wo_sb[:, 2 * DIM :], woT[:, 2 * DIM :])
        # scalar ring, gated: a WAR hazard on wq3_sb (its first column is
        # written from xg6 first) forces the DMA to wait for mid-sweep x,
        # keeping the scalar ring off the early critical bandwidth.
        nc.scalar.copy(wq3_sb[:, 0:1], x_tiles[6][:, 0:1])
        nc.scalar.dma_start(wq3_sb[:, : 16 * HEAD_DIM], wqT[:, 3 * DIM : 3 * DIM + 2048])
        nc.scalar.dma_start(wq3_sb[:, 16 * HEAD_DIM :], wqT[:, 3 * DIM + 2048 : 4 * DIM])
        nc.scalar.dma_start(tabs_sb[:], tabs)
        nc.scalar.dma_start(ident_sb[:], ident)
        nc.scalar.dma_start(mask_sb[:], maskb)

        ck_t = tabs_sb[:, 0:T]
        sk_t = tabs_sb[:, T : 2 * T]
        cq_t = tabs_sb[:, 2 * T : 3 * T]
        sq_t = tabs_sb[:, 3 * T : 4 * T]

        kT_sb = const.tile([128, T], QD, tag="kT")
        vT_sb = const.tile([128, T], BF16, tag="vT")
        v_sb = const.tile([128, BSZ * HEAD_DIM], BF16, tag="v")
        oT_sb = const.tile([128, HQ * T], BF16, tag="oT")

        def xslice(j):
            for gi, (j0, j1) in enumerate(XG):
                if j0 <= j < j1:
                    return x_tiles[gi][:, (j - j0) * T : (j - j0 + 1) * T]
            raise AssertionError(j)

        # ---- PE warm-up: dummy matmuls bridge until the first data
        # lands (~12.5us), lifting the HAM clock gate on the way ----
        ps_warm = ps.tile([128, T], F32, tag="ps", name="warm")
        for _ in range(12):
            nc.tensor.matmul(ps_warm[:], warm_w[:], warm_x[:], start=True, stop=True)
        for _ in range(6):
            nc.tensor.matmul(
                ps_warm[:, 0:128], warm_w[:], warm_x[:, 0:128], start=True, stop=True
            )

        def rope(dst_ap, pssrc, ctab, stab, evict=None):
            # evict once on scalar (bf16), then DVE runs in 16-bit mode
            qe = rt.tile([128, T], BF16, tag="qe")
            nc.scalar.copy(qe[:], pssrc[:])
            swp = rt.tile([128, T], BF16, tag="swp")
            nc.vector.tensor_copy(swp[0:64, :], qe[64:128, :])
            nc.vector.tensor_copy(swp[64:128, :], qe[0:64, :])
            prod = rt.tile([128, T], BF16, tag="prod")
            nc.vector.tensor_mul(prod[:], qe[:], ctab)
            nc.vector.tensor_mul(swp[:], swp[:], stab)
            nc.vector.tensor_add(dst_ap, prod[:], swp[:])

        # ---- combined sweep: K, V, Q0, Q1, Q2 accumulate together over
        # the 32 d-tiles, riding the incoming x stream ----
        ps_k = ps.tile([128, T], F32, tag="ps", name="ps_k")
        ps_v = ps.tile([128, T], F32, tag="ps", name="ps_v")
        ps_q = [None] * HQ
        for h in range(3):
            ps_q[h] = ps.tile([128, T], F32, tag="ps", name=f"ps_q{h}")
        for j in range(ND):
            st, sp = (j == 0), (j == ND - 1)
            xr = xslice(j)
            js = slice(j * HEAD_DIM, (j + 1) * HEAD_DIM)
            nc.tensor.matmul(ps_k[:], wk_sb[:, js], xr, start=st, stop=sp)
            nc.tensor.matmul(ps_v[:], wv_sb[:, js], xr, start=st, stop=sp)
            for h in range(3):
                nc.tensor.matmul(ps_q[h][:], wq_tiles[h][:, js], xr, start=st, stop=sp)

        # rope chains drain PSUM on the DVE; vT eviction on scalar
        rope(kT_sb[:], ps_k[:], ck_t, sk_t)
        nc.scalar.copy(vT_sb[:], ps_v[:])
        qts = {}
        for h in range(3):
            qts[h] = qtp.tile([128, T], QD, tag="qT", name=f"qT{h}")
            rope(qts[h][:], ps_q[h][:], cq_t, sq_t)

        # ---- Q3 sweep, interleaved with attention entry ----
        ps_q3 = ps.tile([128, T], F32, tag="ps", name="ps_q3")

        def q3s(j0, j1):
            for j in range(j0, j1):
                nc.tensor.matmul(
                    ps_q3[:],
                    wq3_sb[:, j * HEAD_DIM : (j + 1) * HEAD_DIM],
                    xslice(j),
                    start=(j == 0),
                    stop=(j == ND - 1),
                )

        def v_transpose():
            for b in range(BSZ):
                bs = slice(b * 128, (b + 1) * 128)
                ps_t = ps.tile([128, T], BF16, tag="ps", name=f"ps_vt{b}")
                nc.tensor.transpose(ps_t[:, 0:128], vT_sb[:, bs], ident_sb[:])
                nc.vector.tensor_copy(v_sb[:, bs], ps_t[:, 0:128])

        s_tiles = {}
        den = sm.tile([128, BSZ * HQ], F32, tag="den")
        rden = sm.tile([128, BSZ * HQ], F32, tag="rden")

        def att_scores(h, qt):
            # mask preloaded into PSUM via identity matmul; scores
            # accumulate on top (start=False)
            ps_s = ps.tile([128, T], F32, tag="ps", name=f"ps_s{h}")
            nc.tensor.matmul(ps_s[:], ident_sb[:], mask_sb[:], start=True, stop=False)
            for b in range(BSZ):
                bs = slice(b * 128, (b + 1) * 128)
                nc.tensor.matmul(
                    ps_s[:, bs], qt[:, bs], kT_sb[:, bs], start=False, stop=True
                )
            s_tiles[h] = ps_s

        def att_soft(h):
            # no reduce_max: logits are bounded in the smooth regime.
            # single exp instruction per head; den via reduce_sum on the
            # idle gpsimd engine.
            ps_s = s_tiles[h]
            hh = slice(h * BSZ, (h + 1) * BSZ)
            p_sb = sm.tile([128, T], BF16, tag="p", name=f"p{h}")
            nc.scalar.activation(p_sb[:], ps_s[:], ACTF.Exp)
            # one reduce for all 4 batches via a [128, b, k] view
            nc.vector.reduce_sum(
                den[:, h * BSZ : (h + 1) * BSZ],
                p_sb[:].rearrange("p (b k) -> p b k", b=BSZ),
                axis=AX.X,
            )
            nc.vector.reciprocal(rden[:, hh], den[:, hh])
            for b in range(BSZ):
                bs = slice(b * 128, (b + 1) * 128)
                nc.vector.tensor_scalar_mul(
                    p_sb[:, bs], p_sb[:, bs], rden[:, h * BSZ + b : h * BSZ + b + 1]
                )
            return p_sb

        pts = {}

        def att_ptrans(h, p_sb):
            ps_pt = ps.tile([128, T], BF16, tag="ps", name=f"ps_pt{h}")
            for b in range(BSZ):
                bs = slice(b * 128, (b + 1) * 128)
                nc.tensor.transpose(ps_pt[:, bs], p_sb[:, bs], ident_sb[:])
            pt_sb = sm.tile([128, T], BF16, tag="pt", name=f"pt{h}")
            nc.vector.tensor_copy(pt_sb[:], ps_pt[:])
            pts[h] = pt_sb

        def att_pv(h):
            ps_o = ps.tile([128, T], F32, tag="ps", name=f"ps_o{h}")
            for b in range(BSZ):
                bs = slice(b * 128, (b + 1) * 128)
                nc.tensor.matmul(
                    ps_o[:, bs], v_sb[:, bs], pts[h][:, bs], start=True, stop=True
                )
            if h < 2:
                nc.vector.tensor_copy(oT_sb[:, h * T : (h + 1) * T], ps_o[:])
            else:
                nc.scalar.copy(oT_sb[:, h * T : (h + 1) * T], ps_o[:])

        # ---- wo projection, chunked per d-tile ----
        ps_ys = {}

        def wo_mm(dt, j):
            if j == 0:
                # dt>=8: rotate through the main ps pool too (its att
                # tiles are gone by then), deepening the drain pipeline
                pool = psy if dt < 8 or dt % 2 == 0 else ps
                tg = "psy" if pool is psy else "ps"
                ps_ys[dt] = pool.tile([128, T], F32, tag=tg, name=f"ps_y{dt}")
            nc.tensor.matmul(
                ps_ys[dt][:],
                wo_sb[:, j * DIM + dt * 128 : j * DIM + (dt + 1) * 128],
                oT_sb[:, j * T : (j + 1) * T],
                start=(j == 0),
                stop=(j == HQ - 1),
            )

        # y staging: groups of 4 d-tiles (last two groups of 2)
        YG = [(0, 4), (4, 8), (8, 12), (12, 16), (16, 20), (20, 24),
              (24, 28), (28, 30), (30, 32)]
        ystage = {}

        def y_drain(dt):
            for g0, g1 in YG:
                if g0 <= dt < g1:
                    break
            if dt == g0:
                ystage[g0] = yp.tile([128, (g1 - g0) * T], BF16, tag="y", name=f"y{g0}")
            # split the drain across vector+scalar so its latency stays
            # below the 4-matmul chunk time (psy ping-pong never stalls)
            dst = ystage[g0][:, (dt - g0) * T : (dt - g0 + 1) * T]
            H = T // 2
            nc.vector.tensor_copy(dst[:, 0:H], ps_ys[dt][:, 0:H])
            nc.scalar.copy(dst[:, H:T], ps_ys[dt][:, H:T])
            del ps_ys[dt]
            if dt == g1 - 1:
                nc.sync.dma_start(yT[:, g0 * T : g1 * T], ystage[g0][:])

        # ---- attention entry, interleaved with the q3 sweep ----
        q3s(0, 8)
        v_transpose()
        att_scores(0, qts[0])
        p0 = att_soft(0)
        q3s(8, 16)
        att_scores(1, qts[1])
        p1 = att_soft(1)
        q3s(16, 24)
        att_scores(2, qts[2])
        q3s(24, ND)
        qts[3] = qtp.tile([128, T], QD, tag="qT", name="qT3")
        rope(qts[3][:], ps_q3[:], cq_t, sq_t)
        p2 = att_soft(2)
        att_ptrans(0, p0)
        att_pv(0)
        att_ptrans(1, p1)
        att_pv(1)
        att_scores(3, qts[3])
        p3 = att_soft(3)
        wo_mm(0, 0)
        wo_mm(1, 0)
        wo_mm(0, 1)
        wo_mm(1, 1)
        att_ptrans(2, p2)
        att_pv(2)
        wo_mm(0, 2)
        wo_mm(1, 2)
        att_ptrans(3, p3)
        att_pv(3)
        wo_mm(0, 3)
        wo_mm(1, 3)
        y_drain(0)
        y_drain(1)
        for dt in range(2, ND):
            for j in range(HQ):
                wo_mm(dt, j)
            y_drain(dt)

    nc.compile()
    return nc


def _build_nc_robust():
    """Original proven build for the fp32r (winner-take-all) path."""
    XD = F32R
    QD = F32
    TD = F32
    nc = bacc.Bacc(
        "TRN2",
        target_bir_lowering=False,
        debug=False,
        enable_asserts=False,
        num_devices=NCORES,
    )
    xT = nc.dram_tensor("xT", [128, ND * T], XD, kind="ExternalInput").ap()
    wqT = nc.dram_tensor("wqT", [128, HQ * ND * HEAD_DIM], XD, kind="ExternalInput").ap()
    wkT = nc.dram_tensor("wkT", [128, ND * HEAD_DIM], XD, kind="ExternalInput").ap()
    wvT = nc.dram_tensor("wvT", [128, ND * HEAD_DIM], XD, kind="ExternalInput").ap()
    woT = nc.dram_tensor("woT", [128, HQ * DIM], BF16, kind="ExternalInput").ap()
    mask1 = nc.dram_tensor("mask1", [128, 128], F32, kind="ExternalInput").ap()
    cq = nc.dram_tensor("cq", [128, T], TD, kind="ExternalInput").ap()
    sq = nc.dram_tensor("sq", [128, T], TD, kind="ExternalInput").ap()
    ck = nc.dram_tensor("ck", [128, T], TD, kind="ExternalInput").ap()
    sk = nc.dram_tensor("sk", [128, T], TD, kind="ExternalInput").ap()
    ident = nc.dram_tensor("ident", [128, 128], BF16, kind="ExternalInput").ap()
    yT = nc.dram_tensor("yT", [DIM, T], BF16, kind="ExternalOutput").ap()

    with tile.TileContext(nc) as tc, ExitStack() as ctx:
        const = ctx.enter_context(tc.tile_pool(name="const", bufs=1))
        wp = ctx.enter_context(tc.tile_pool(name="wp", bufs=4))
        qtp = ctx.enter_context(tc.tile_pool(name="qtp", bufs=4))
        rt = ctx.enter_context(tc.tile_pool(name="rt", bufs=1))
        sm = ctx.enter_context(tc.tile_pool(name="sm", bufs=2))
        yp = ctx.enter_context(tc.tile_pool(name="yp", bufs=2))
        ps = ctx.enter_context(tc.tile_pool(name="ps", bufs=7, space=PSUM))
        wps = ctx.enter_context(tc.tile_pool(name="wps", bufs=1, space=PSUM))

        warm_w = const.tile([128, 128], BF16, tag="warm_w")
        nc.gpsimd.memset(warm_w[:], 0.0)
        warm_x = const.tile([128, T], BF16, tag="warm_x")
        nc.gpsimd.memset(warm_x[:], 0.0)
        ps_warm = wps.tile([128, T], F32, tag="wps")
        for _ in range(10):
            nc.tensor.matmul(ps_warm[:], warm_w[:], warm_x[:], start=True, stop=True)

        wk_sb = wp.tile([128, ND * HEAD_DIM], XD, tag="w", name="wk")
        nc.sync.dma_start(wk_sb[:], wkT)
        wv_sb = wp.tile([128, ND * HEAD_DIM], XD, tag="w", name="wv")
        nc.scalar.dma_start(wv_sb[:], wvT)

        XGROUPS = [2, 2, 2, 2, 4, 4, 4, 4, 4, 4]
        x_tiles = [None] * len(XGROUPS)
        xg_col = []
        j0 = 0
        for gi, gd in enumerate(XGROUPS):
            xg_col.append((j0, gd))
            j0 += gd

        def load_x(gi, eng):
            j0, gd = xg_col[gi]
            xg = const.tile([128, gd * T], XD, tag=f"x{gi}", name=f"x{gi}")
            eng.dma_start(xg[:], xT[:, j0 * T : (j0 + gd) * T])
            x_tiles[gi] = xg

        wq_tiles = [None] * HQ

        def load_wq(h, eng):
            wqt = wp.tile([128, ND * HEAD_DIM], XD, tag="w", name=f"wq{h}")
            eng.dma_start(wqt[:], wqT[:, h * DIM : (h + 1) * DIM])
            wq_tiles[h] = wqt

        load_x(0, nc.sync)
        load_wq(0, nc.scalar)
        load_wq(1, nc.sync)
        load_x(1, nc.scalar)
        load_wq(2, nc.sync)
        load_x(2, nc.scalar)
        load_wq(3, nc.sync)
        ident_sb = const.tile([128, 128], BF16, tag="ident")
        nc.scalar.dma_start(ident_sb[:], ident)
        ck_sb = const.tile([128, T], TD, tag="ck")
        nc.scalar.dma_start(ck_sb[:], ck)
        sk_sb = const.tile([128, T], TD, tag="sk")
        nc.scalar.dma_start(sk_sb[:], sk)
        cq_sb = const.tile([128, T], TD, tag="cq")
        nc.scalar.dma_start(cq_sb[:], cq)
        sq_sb = const.tile([128, T], TD, tag="sq")
        nc.scalar.dma_start(sq_sb[:], sq)
        mask_sb = const.tile([128, 128], F32, tag="mask")
        nc.scalar.dma_start(mask_sb[:], mask1)
        for gi in range(3, len(XGROUPS)):
            load_x(gi, nc.scalar if gi % 2 == 0 else nc.sync)
        wo_sb = const.tile([128, HQ * DIM], BF16, tag="wo")
        nc.sync.dma_start(wo_sb[:, : 2 * DIM], woT[:, : 2 * DIM])
        nc.scalar.dma_start(wo_sb[:, 2 * DIM :], woT[:, 2 * DIM :])

        kT_sb = const.tile([128, T], QD, tag="kT")
        vT_sb = const.tile([128, T], BF16, tag="vT")
        v_sb = const.tile([128, BSZ * HEAD_DIM], BF16, tag="v")
        oT_sb = const.tile([128, HQ * T], BF16, tag="oT")

        def xslice(j):
            gi = 0
            j0 = 0
            for i, (jj0, gd) in enumerate(xg_col):
                if jj0 <= j < jj0 + gd:
                    gi, j0 = i, jj0
                    break
            return x_tiles[gi][:, (j - j0) * T : (j - j0 + 1) * T]

        def rope(dst_ap, pssrc, ctab, stab):
            swp = rt.tile([128, T], F32, tag="swp")
            nc.scalar.copy(swp[0:64, :], pssrc[64:128, :])
            nc.scalar.copy(swp[64:128, :], pssrc[0:64, :])
            prod = rt.tile([128, T], F32, tag="prod")
            nc.vector.tensor_mul(prod[:], pssrc[:], ctab)
            nc.vector.tensor_mul(swp[:], swp[:], stab)
            nc.vector.tensor_add(dst_ap, prod[:], swp[:])

        ps_k = ps.tile([128, T], F32, tag="ps")
        ps_v = ps.tile([128, T], F32, tag="ps")
        ps_q = [None] * HQ
        NSW = 2
        for h in range(NSW):
            ps_q[h] = ps.tile([128, T], F32, tag="ps", name=f"ps_q{h}")
        for j in range(ND):
            st, sp = (j == 0), (j == ND - 1)
            xr = xslice(j)
            js = slice(j * HEAD_DIM, (j + 1) * HEAD_DIM)
            nc.tensor.matmul(ps_k[:], wk_sb[:, js], xr, start=st, stop=sp)
            nc.tensor.matmul(ps_v[:], wv_sb[:, js], xr, start=st, stop=sp)
            for h in range(NSW):
                nc.tensor.matmul(ps_q[h][:], wq_tiles[h][:, js], xr, start=st, stop=sp)

        rope(kT_sb[:], ps_k[:], ck_sb[:], sk_sb[:])
        qts = {}
        for h in range(NSW):
            qts[h] = qtp.tile([128, T], QD, tag="qT", name=f"qT{h}")
            rope(qts[h][:], ps_q[h][:], cq_sb[:], sq_sb[:])

        def q_sweep(h):
            ps_qh = ps.tile([128, T], F32, tag="ps", name=f"ps_q{h}")
            for j in range(ND):
                st, sp = (j == 0), (j == ND - 1)
                js = slice(j * HEAD_DIM, (j + 1) * HEAD_DIM)
                nc.tensor.matmul(
                    ps_qh[:], wq_tiles[h][:, js], xslice(j), start=st, stop=sp
                )
            qt = qtp.tile([128, T], QD, tag="qT", name=f"qT{h}")
            rope(qt[:], ps_qh[:], cq_sb[:], sq_sb[:])
            return qt

        def keep_warm(n=2):
            for _ in range(n):
                nc.tensor.matmul(
                    ps_warm[:], warm_w[:], warm_x[:], start=True, stop=True
                )

        def att_scores(h, qt):
            ps_s = ps.tile([128, T], F32, tag="ps", name=f"ps_s{h}")
            for b in range(BSZ):
                bs = slice(b * 128, (b + 1) * 128)
                nc.tensor.matmul(
                    ps_s[:, bs], qt[:, bs], kT_sb[:, bs], start=True, stop=True
                )
            s_sb = sm.tile([128, T], F32, tag="s", name=f"s{h}")
            nmx = sm.tile([128, BSZ], F32, tag="nmx", name=f"nmx{h}")
            den = sm.tile([128, BSZ], F32, tag="den", name=f"den{h}")
            rden = sm.tile([128, BSZ], F32, tag="rden", name=f"rden{h}")
            p_sb = sm.tile([128, T], BF16, tag="p", name=f"p{h}")
            for b in range(BSZ):
                bs = slice(b * 128, (b + 1) * 128)
                nc.vector.tensor_add(s_sb[:, bs], ps_s[:, bs], mask_sb[:])
                nc.vector.reduce_max(
                    nmx[:, b : b + 1], s_sb[:, bs], axis=AX.X, negate=True
                )
                nc.scalar.activation(
                    p_sb[:, bs],
                    s_sb[:, bs],
                    ACTF.Exp,
                    bias=nmx[:, b : b + 1],
                    accum_out=den[:, b : b + 1],
                )
            nc.vector.reciprocal(rden[:], den[:])
            for b in range(BSZ):
                bs = slice(b * 128, (b + 1) * 128)
                nc.vector.tensor_scalar_mul(p_sb[:, bs], p_sb[:, bs], rden[:, b : b + 1])
            return p_sb

        def att_pv(h, p_sb):
            ps_pt = ps.tile([128, T], BF16, tag="ps", name=f"ps_pt{h}")
            for b in range(BSZ):
                bs = slice(b * 128, (b + 1) * 128)
                nc.tensor.transpose(ps_pt[:, bs], p_sb[:, bs], ident_sb[:])
            pt_sb = sm.tile([128, T], BF16, tag="pt", name=f"pt{h}")
            nc.scalar.copy(pt_sb[:], ps_pt[:])
            ps_o = ps.tile([128, T], F32, tag="ps", name=f"ps_o{h}")
            for b in range(BSZ):
                bs = slice(b * 128, (b + 1) * 128)
                nc.tensor.matmul(
                    ps_o[:, bs], v_sb[:, bs], pt_sb[:, bs], start=True, stop=True
                )
            if h % 2 == 0:
                nc.vector.tensor_copy(oT_sb[:, h * T : (h + 1) * T], ps_o[:])
            else:
                nc.scalar.copy(oT_sb[:, h * T : (h + 1) * T], ps_o[:])

        probs = {}
        qts[2] = q_sweep(2)
        nc.scalar.copy(vT_sb[:], ps_v[:])
        for b in range(BSZ):
            bs = slice(b * 128, (b + 1) * 128)
            ps_t = ps.tile([128, T], BF16, tag="ps")
            nc.tensor.transpose(ps_t[:, 0:128], vT_sb[:, bs], ident_sb[:])
            nc.vector.tensor_copy(v_sb[:, bs], ps_t[:, 0:128])
        probs[0] = att_scores(0, qts[0])
        probs[1] = att_scores(1, qts[1])
        qts[3] = q_sweep(3)
        att_pv(0, probs[0])
        probs[2] = att_scores(2, qts[2])
        att_pv(1, probs[1])
        keep_warm(2)
        probs[3] = att_scores(3, qts[3])
        att_pv(2, probs[2])
        keep_warm(2)
        att_pv(3, probs[3])

        for dt in range(ND):
            ps_y = ps.tile([128, T], F32, tag="ps", name=f"ps_y{dt}")
            for j in range(HQ):
                nc.tensor.matmul(
                    ps_y[:],
                    wo_sb[:, j * DIM + dt * 128 : j * DIM + (dt + 1) * 128],
                    oT_sb[:, j * T : (j + 1) * T],
                    start=(j == 0),
                    stop=(j == HQ - 1),
                )
            y_sb = yp.tile([128, T], BF16, tag="y", name=f"y{dt}")
            if dt % 2 == 0:
                nc.vector.tensor_copy(y_sb[:], ps_y[:])
                nc.sync.dma_start(yT[dt * 128 : (dt + 1) * 128, :], y_sb[:])
            else:
                nc.scalar.copy(y_sb[:], ps_y[:])
                nc.scalar.dma_start(yT[dt * 128 : (dt + 1) * 128, :], y_sb[:])

    nc.compile()
    return nc


def get_nc(fast: bool):
    key = "nc_fast" if fast else "nc_robust"
    if key not in _STATE:
        _STATE[key] = _build_nc_fast() if fast else _build_nc_robust()
    return _STATE[key]


def _prep_in_maps(x, wq, wk, wv, wo, freqs_cos, freqs_sin, mask, fast):
    f32 = np.float32
    bf16 = ml_dtypes.bfloat16
    xd = bf16 if fast else f32
    x = np.asarray(x, f32)
    wq = np.asarray(wq, f32)
    wk = np.asarray(wk, f32)
    wv = np.asarray(wv, f32)
    wo = np.asarray(wo, f32)
    fc = np.asarray(freqs_cos, f32)
    fs = np.asarray(freqs_sin, f32)
    mask = np.asarray(mask, f32)

    # even features first, then odd: (2i, 2i+1) pairs -> (i, i+64)
    perm = np.concatenate([np.arange(0, HEAD_DIM, 2), np.arange(1, HEAD_DIM, 2)])
    wqp = wq.reshape(N_HEADS, HEAD_DIM, DIM)[:, perm, :].reshape(DIM, DIM)
    wkp = wk.reshape(N_KV_HEADS, HEAD_DIM, DIM)[:, perm, :].reshape(
        N_KV_HEADS * HEAD_DIM, DIM
    )

    def sw_x(xmat):  # [T, DIM] -> [128, ND*T]: (p, j*T + t) = x[t, j*128+p]
        return np.ascontiguousarray(
            xmat.T.reshape(ND, 128, T).transpose(1, 0, 2).reshape(128, ND * T)
        )

    def sw_w(wmat):  # [E(128), DIM] -> [128, ND*E]: (p, j*E + e) = w[e, j*128+p]
        E = wmat.shape[0]
        return np.ascontiguousarray(
            wmat.T.reshape(ND, 128, E).transpose(1, 0, 2).reshape(128, ND * E)
        )

    xT = sw_x(x.reshape(T, DIM)).astype(xd)
    C0 = np.vstack([fc.T, fc.T])  # [128, 128]: row p -> cos[t, p % 64]
    S0 = np.vstack([-fs.T, fs.T])
    td = bf16 if fast else f32
    cq = np.tile(C0 * SCALE, (1, BSZ))
    sq = np.tile(S0 * SCALE, (1, BSZ))
    ck = np.tile(C0, (1, BSZ))
    sk = np.tile(S0, (1, BSZ))
    mask1 = np.ascontiguousarray(mask[0, 0])
    ident = np.eye(128, dtype=bf16)

    in_maps = []
    for c in range(NCORES):
        qrows = slice(c * EQ, (c + 1) * EQ)
        krows = slice(c * HEAD_DIM, (c + 1) * HEAD_DIM)
        wq_heads = [
            sw_w(wqp[c * EQ + h * HEAD_DIM : c * EQ + (h + 1) * HEAD_DIM, :])
            for h in range(HQ)
        ]
        # wo: (p, j*DIM + dout) = wo[dout, c*EQ + j*128 + p]
        wo_sw = np.ascontiguousarray(
            wo[:, qrows].T.reshape(HQ, 128, DIM).transpose(1, 0, 2).reshape(128, HQ * DIM)
        ).astype(bf16)
        im = {
            "xT": xT,
            "wqT": np.ascontiguousarray(np.concatenate(wq_heads, axis=1)).astype(xd),
            "wkT": sw_w(wkp[krows, :]).astype(xd),
            "wvT": sw_w(wv[krows, :]).astype(xd),
            "woT": wo_sw,
            "ident": ident,
        }
        if fast:
            im["tabs"] = np.ascontiguousarray(
                np.concatenate([ck, sk, cq, sq], axis=1)
            ).astype(td)
            im["maskb"] = np.ascontiguousarray(np.tile(mask1, (1, BSZ))).astype(bf16)
        else:
            im["mask1"] = mask1
            im["cq"] = np.ascontiguousarray(cq).astype(td)
            im["sq"] = np.ascontiguousarray(sq).astype(td)
            im["ck"] = np.ascontiguousarray(ck).astype(td)
            im["sk"] = np.ascontiguousarray(sk).astype(td)
        in_maps.append(im)
    return in_maps


def _pick_fast(x, wq):
    """bf16 q/k only when softmax logits are smooth (score sigma small).

    score_sigma ~= std(x) * std(wq) * sqrt(DIM * HEAD_DIM) * SCALE. In the
    winner-take-all regime (sigma >> 1) bf16 rounding flips argmaxes, so use
    the fp32r path there.
    """
    sx = float(np.asarray(x, np.float32).std())
    sw = float(np.asarray(wq, np.float32).std())
    sigma = sx * sw * math.sqrt(DIM * HEAD_DIM) * SCALE
    return sigma < 8.0


def kernel(
    x,
    wq,
    wk,
    wv,
    wo,
    cache_k,
    cache_v,
    freqs_cos,
    freqs_sin,
    mask,
    start_pos,
    *,
    trace=False,
    trace_kwargs=None,
):
    global LAST_RESULT
    sp = int(np.asarray(start_pos))
    assert sp == 0, f"kernel specialized for start_pos=0, got {sp}"

    fast = _pick_fast(x, wq)
    in_maps = _prep_in_maps(x, wq, wk, wv, wo, freqs_cos, freqs_sin, mask, fast)
    nc = get_nc(fast)
    res = run_bass_kernel_spmd(
        nc,
        in_maps,
        core_ids=list(range(NCORES)),
        trace=trace,
        **(trace_kwargs or {}),
    )
    LAST_RESULT = res
    if fast:
        acc = np.zeros((128, ND * T), np.float32)
        for c in range(NCORES):
            acc += res.results[c]["yT"].astype(np.float32)
        # unswizzle: y[dt*128+p, t] = yT[p, dt*T+t]
        yfull = acc.reshape(128, ND, T).transpose(1, 0, 2).reshape(DIM, T)
    else:
        yfull = np.zeros((DIM, T), np.float32)
        for c in range(NCORES):
            yfull += res.results[c]["yT"].astype(np.float32)
    return np.ascontiguousarray(yfull.T).reshape(BSZ, SEQLEN, DIM)
